# revision 2
# baseline (speedup 1.0000x reference)
"""CrossLinearAttention Trainium2 kernel v2 (8 NeuronCores, SPMD).

b=4, n1=n2=8192, dim=256, 8 heads x 64. Flat rows 32768 -> 4096/core.
Each core's shard lies in one batch element (b = c // 2); partial
k^T v (dotsT) AllReduced between core pairs.

v2 vs baseline:
  - bf16 matmul operands everywhere (tolerance 2e-2 >> bf16 error).
  - Host pre-transposes x, z -> xT, zT [256, rows]; projections use
    xT/zT chunks as the stationary operand: no PE transposes of
    activations, no PSUM->SBUF copies on the z side.
  - Within-head feature order permuted to [x1st16, y1st16, x2nd16,
    y2nd16] so rotate-half is a uniform +-32 column shift (single
    strided view); sin tables sign-baked.
  - Rotary add fused into dots PSUM accumulation: dotsT accumulates
    (v2^T kc) + (v2^T ks) over all tiles.
  - dots -> output projection fused: W2 = blkdiag(dotsT) @ Wo^T
    computed once on device after AllReduce; y^T = W2^T-chunks @ qT,
    DMA'd from PSUM to DRAM; host un-transposes and adds bias.
"""

import sys

sys.path.insert(0, "/opt/trn_rl_repo")

from contextlib import ExitStack

import ml_dtypes
import numpy as np

import concourse.bass as bass
import concourse.tile as tile
from concourse import bacc, mybir
from concourse.bass import ts
from concourse.bass_utils import run_bass_kernel_spmd
from concourse.masks import make_identity

F32 = mybir.dt.float32
BF16 = mybir.dt.bfloat16
ALU = mybir.AluOpType
AX = mybir.AxisListType
AF = mybir.ActivationFunctionType

B, N1, DIM = 4, 8192, 256
H, DH = 8, 64
INNER = H * DH  # 512
NCORES = 8
ROWS = (B * N1) // NCORES  # 4096 rows per core
NT = ROWS // 128  # 32 tiles of 128 rows
EPS = 1e-5
BF = ml_dtypes.bfloat16

_nc_cache = {}


def _view(ap, offset, dims):
    """AP view with explicit free dims; keeps the partition dim."""
    return bass.AP(
        tensor=ap.tensor,
        offset=ap.offset + offset,
        ap=[list(ap.ap[0])] + [list(d) for d in dims],
    )


def build_nc():
    if "nc" in _nc_cache:
        return _nc_cache["nc"]
    nc = bacc.Bacc(trn_type="TRN2", num_devices=NCORES, debug=False)

    xT_d = nc.dram_tensor("xT", [DIM, ROWS], BF16, kind="ExternalInput").ap()
    zT_d = nc.dram_tensor("zT", [DIM, ROWS], BF16, kind="ExternalInput").ap()
    wq_d = nc.dram_tensor("wq", [DIM, INNER], BF16, kind="ExternalInput").ap()
    wkv_d = nc.dram_tensor("wkv", [DIM, 2 * INNER], BF16, kind="ExternalInput").ap()
    wo_d = nc.dram_tensor("wo", [INNER, DIM], BF16, kind="ExternalInput").ap()
    cq_d = nc.dram_tensor("cq", [ROWS, DH], BF16, kind="ExternalInput").ap()
    sq_d = nc.dram_tensor("sq", [ROWS, DH], BF16, kind="ExternalInput").ap()
    ck_d = nc.dram_tensor("ck", [ROWS, DH], BF16, kind="ExternalInput").ap()
    sk_d = nc.dram_tensor("sk", [ROWS, DH], BF16, kind="ExternalInput").ap()
    yT_d = nc.dram_tensor("yT", [DIM, ROWS], BF16, kind="ExternalOutput").ap()
    cc_in = nc.dram_tensor("cc_in", [128, INNER], F32)
    cc_out = nc.dram_tensor("cc_out", [128, INNER], F32)

    with tile.TileContext(nc) as tc, ExitStack() as ctx:
        consts = ctx.enter_context(tc.tile_pool(name="consts", bufs=1))

        ident_f = consts.tile([128, 128], F32)
        make_identity(nc, ident_f)
        ident = consts.tile([128, 128], BF16)
        nc.vector.tensor_copy(ident, ident_f)
        eps_t = consts.tile([128, 1], F32)
        nc.vector.memset(eps_t, EPS)

        # phase-A-critical consts first on the sync queue; B-side consts
        # ride other engine queues so they stream in behind A's compute
        zT_sb, xT_sb, wq_t, wkv_t = [], [], [], []
        for c in range(2):
            t = consts.tile([128, 2 * INNER], BF16, tag=f"wkv{c}")
            nc.sync.dma_start(t, wkv_d[ts(c, 128), :])
            wkv_t.append(t)
        NCH = 4
        CH = ROWS // NCH
        for c in range(2):
            zT_sb.append(
                [consts.tile([128, CH], BF16, tag=f"zT{c}_{j}", name=f"zT{c}_{j}")
                 for j in range(NCH)]
            )
            xT_sb.append(
                [consts.tile([128, CH], BF16, tag=f"xT{c}_{j}", name=f"xT{c}_{j}")
                 for j in range(NCH)]
            )
        for j in range(NCH):
            for c in range(2):
                nc.sync.dma_start(zT_sb[c][j], zT_d[ts(c, 128), ts(j, CH)])
        for c in range(2):
            t = consts.tile([128, INNER], BF16, tag=f"wq{c}")
            nc.scalar.dma_start(t, wq_d[ts(c, 128), :])
            wq_t.append(t)
        for j in range(NCH):
            for c in range(2):
                nc.scalar.dma_start(xT_sb[c][j], xT_d[ts(c, 128), ts(j, CH)])

        def zchunk(c, t):
            return zT_sb[c][t // (NT // NCH)][:, ts(t % (NT // NCH), 128)]

        def xchunk(c, t):
            return xT_sb[c][t // (NT // NCH)][:, ts(t % (NT // NCH), 128)]
        wo_t = []
        for p in range(4):
            t = consts.tile([128, DIM], BF16, tag=f"wo{p}")
            nc.gpsimd.dma_start(t, wo_d[ts(p, 128), :])
            wo_t.append(t)
        # qT staging: tile t occupies cols [512t, 512t+512), f-chunk f at
        # [512t+128f, 512t+128f+128). yT matmuls read strided views.
        qTb = consts.tile([128, NT * INNER], BF16)

        # ---------------- phase A: z side -> dotsT ----------------
        with tc.tile_pool(name="kv_pp", bufs=2, space="PSUM") as kv_pp, \
             tc.tile_pool(name="dots_pp", bufs=1, space="PSUM") as dots_pp, \
             tc.tile_pool(name="tabsA", bufs=5) as tabsA, \
             tc.tile_pool(name="elemA", bufs=5) as elemA, \
             tc.tile_pool(name="statsA", bufs=6) as statsA:

            dots_ps = []
            for p in range(4):
                dots_ps.append(
                    dots_pp.tile([128, 512], F32, tag=f"dots{p}", name=f"dots{p}")
                )

            for t in range(NT):
                ck_t = tabsA.tile([128, DH], BF16, tag="ck")
                nc.sync.dma_start(ck_t, ck_d[ts(t, 128), :])
                sk_t = tabsA.tile([128, DH], BF16, tag="sk")
                nc.sync.dma_start(sk_t, sk_d[ts(t, 128), :])

                kv_ps = kv_pp.tile([128, 2 * INNER], F32, tag="kv")
                for c in range(2):
                    nc.tensor.matmul(
                        kv_ps[:, 0:INNER],
                        zchunk(c, t),
                        wkv_t[c][:, 0:INNER],
                        start=(c == 0),
                        stop=(c == 1),
                    )
                for c in range(2):
                    nc.tensor.matmul(
                        kv_ps[:, INNER : 2 * INNER],
                        zchunk(c, t),
                        wkv_t[c][:, INNER : 2 * INNER],
                        start=(c == 0),
                        stop=(c == 1),
                    )

                # single PSUM reader: convert then everything reads SBUF bf16
                kvb_t = elemA.tile([128, 2 * INNER], BF16, tag="kvb")
                nc.scalar.copy(kvb_t, kv_ps)

                sq_t = elemA.tile([128, 2 * INNER], BF16, tag="sq")
                nc.scalar.activation(sq_t[:, 0:INNER], kvb_t[:, 0:INNER], AF.Square)
                nc.vector.tensor_tensor(
                    sq_t[:, INNER : 2 * INNER],
                    kvb_t[:, INNER : 2 * INNER],
                    kvb_t[:, INNER : 2 * INNER],
                    op=ALU.mult,
                )
                red_t = statsA.tile([128, 16], F32, tag="red")
                nc.vector.reduce_sum(
                    red_t, _view(sq_t, 0, [[64, 16], [1, 64]]), axis=AX.X
                )
                std_t = statsA.tile([128, 16], F32, tag="std")
                nc.scalar.activation(
                    std_t, red_t, AF.Sqrt, scale=1.0 / DH, bias=eps_t[:, 0:1]
                )
                rstd_t = statsA.tile([128, 16], F32, tag="rstd")
                nc.vector.reciprocal(rstd_t, std_t)
                rkv_t = statsA.tile([128, 8], F32, tag="rkv")
                nc.vector.tensor_tensor(
                    rkv_t, rstd_t[:, 0:8], rstd_t[:, 8:16], op=ALU.mult
                )

                kc_t = elemA.tile([128, INNER], BF16, tag="kc")
                nc.vector.tensor_tensor(
                    kc_t,
                    kvb_t[:, 0:INNER],
                    _view(ck_t, 0, [[0, 8], [1, 64]]),
                    op=ALU.mult,
                )
                ks_t = elemA.tile([128, INNER], BF16, tag="ks")
                nc.gpsimd.tensor_tensor(
                    _view(ks_t, 0, [[64, 8], [32, 2], [1, 32]]),
                    _view(kvb_t, 32, [[64, 8], [-32, 2], [1, 32]]),
                    _view(sk_t, 0, [[0, 8], [32, 2], [1, 32]]),
                    op=ALU.mult,
                )
                v2_t = elemA.tile([128, INNER], BF16, tag="v2")
                nc.vector.tensor_tensor(
                    v2_t[:, 0:256],
                    kvb_t[:, INNER : INNER + 256],
                    _view(rkv_t, 0, [[1, 4], [0, 64]]),
                    op=ALU.mult,
                )
                nc.gpsimd.tensor_tensor(
                    v2_t[:, 256:512],
                    kvb_t[:, INNER + 256 : 2 * INNER],
                    _view(rkv_t, 4, [[1, 4], [0, 64]]),
                    op=ALU.mult,
                )

                for p in range(4):
                    nc.tensor.matmul(
                        dots_ps[p][:, 0:128],
                        v2_t[:, ts(p, 128)],
                        kc_t[:, ts(p, 128)],
                        start=(t == 0),
                        stop=False,
                    )
                    nc.tensor.matmul(
                        dots_ps[p][:, 0:128],
                        v2_t[:, ts(p, 128)],
                        ks_t[:, ts(p, 128)],
                        start=False,
                        stop=(t == NT - 1),
                    )

            dotsT_sb = consts.tile([128, INNER], F32)
            for p in range(4):
                nc.vector.tensor_copy(dotsT_sb[:, ts(p, 128)], dots_ps[p][:, 0:128])

        # collective rides the gpsimd queue (idle during phase B) so its
        # DMAs don't block phase B's table loads on the sync queue
        nc.gpsimd.dma_start(cc_in.ap(), dotsT_sb)
        nc.gpsimd.collective_compute(
            "AllReduce",
            ALU.add,
            replica_groups=[[0, 1], [2, 3], [4, 5], [6, 7]],
            ins=[cc_in.ap()],
            outs=[cc_out.ap()],
        )
        dots_rd = consts.tile([128, INNER], F32)
        nc.gpsimd.dma_start(dots_rd, cc_out.ap())

        # ---------------- phase B: q side (overlaps AllReduce) ----------------
        with tc.tile_pool(name="q_pp", bufs=2, space="PSUM") as q_pp, \
             tc.tile_pool(name="qt_pp", bufs=2, space="PSUM") as qt_pp, \
             tc.tile_pool(name="tabsB", bufs=5) as tabsB, \
             tc.tile_pool(name="elemB", bufs=5) as elemB:

            for t in range(NT):
                cq_t = tabsB.tile([128, DH], BF16, tag="cq")
                nc.sync.dma_start(cq_t, cq_d[ts(t, 128), :])
                sq2_t = tabsB.tile([128, DH], BF16, tag="sq2")
                nc.sync.dma_start(sq2_t, sq_d[ts(t, 128), :])

                q_ps = q_pp.tile([128, INNER], F32, tag="q")
                for c in range(2):
                    nc.tensor.matmul(
                        q_ps,
                        xchunk(c, t),
                        wq_t[c],
                        start=(c == 0),
                        stop=(c == 1),
                    )
                qb_t = elemB.tile([128, INNER], BF16, tag="qb")
                nc.scalar.copy(qb_t, q_ps)
                qc_t = elemB.tile([128, INNER], BF16, tag="qc")
                nc.vector.tensor_tensor(
                    qc_t, qb_t, _view(cq_t, 0, [[0, 8], [1, 64]]), op=ALU.mult
                )
                qs_t = elemB.tile([128, INNER], BF16, tag="qs")
                nc.vector.tensor_tensor(
                    _view(qs_t, 0, [[64, 8], [32, 2], [1, 32]]),
                    _view(qb_t, 32, [[64, 8], [-32, 2], [1, 32]]),
                    _view(sq2_t, 0, [[0, 8], [32, 2], [1, 32]]),
                    op=ALU.mult,
                )
                qr_t = elemB.tile([128, INNER], BF16, tag="qr")
                nc.vector.tensor_tensor(qr_t, qc_t, qs_t, op=ALU.add)

                qT_ps = qt_pp.tile([128, INNER], BF16, tag="qT")
                for f in range(4):
                    nc.tensor.transpose(
                        qT_ps[:, ts(f, 128)], qr_t[:, ts(f, 128)], ident
                    )
                nc.scalar.copy(qTb[:, ts(t, INNER)], qT_ps)

        # ---------------- W2 = blkdiag(dotsT) @ WoT ----------------
        blk = consts.tile([128, INNER], BF16)
        nc.vector.memset(blk, 0.0)
        nc.vector.tensor_copy(
            _view(blk[0:64, 0:1], 0, [[128, 4], [1, 64]]),
            _view(dots_rd[0:64, 0:1], 0, [[128, 4], [1, 64]]),
        )
        nc.vector.tensor_copy(
            _view(blk[64:128, 0:1], 64, [[128, 4], [1, 64]]),
            _view(dots_rd[64:128, 0:1], 64, [[128, 4], [1, 64]]),
        )
        W2b = []
        with tc.tile_pool(name="w2_pp", bufs=1, space="PSUM") as w2_pp:
            for p in range(4):
                w2_ps = w2_pp.tile([128, 512], F32, tag=f"w2{p}", name=f"w2{p}")
                nc.tensor.matmul(
                    w2_ps[:, 0:DIM], blk[:, ts(p, 128)], wo_t[p],
                    start=True, stop=True,
                )
                w2b = consts.tile([128, DIM], BF16, tag=f"W2b{p}")
                nc.scalar.copy(w2b, w2_ps[:, 0:DIM])
                W2b.append(w2b)

            # ---------------- yT = W2^T-chunks @ qTb ----------------
            with tc.tile_pool(name="y_pp", bufs=4, space="PSUM") as y_pp, \
                 tc.tile_pool(name="yout", bufs=4) as yout:
                for r in range(ROWS // 512):
                    for o in range(2):
                        yT_ps = y_pp.tile([128, 512], F32, tag="yT")
                        for f in range(4):
                            nc.tensor.matmul(
                                yT_ps,
                                W2b[f][:, ts(o, 128)],
                                _view(qTb, r * 4 * INNER + f * 128,
                                      [[INNER, 4], [1, 128]]),
                                start=(f == 0),
                                stop=(f == 3),
                            )
                        yT_sb = yout.tile([128, 512], BF16, tag="ysb")
                        if o == 0:
                            nc.vector.tensor_copy(yT_sb, yT_ps)
                        else:
                            nc.scalar.copy(yT_sb, yT_ps)
                        nc.sync.dma_start(yT_d[ts(o, 128), ts(r, 512)], yT_sb)

    nc.compile()
    _nc_cache["nc"] = nc
    return nc


PERM64 = np.concatenate(
    [np.arange(0, 16), np.arange(32, 48), np.arange(16, 32), np.arange(48, 64)]
)
PERM = (np.arange(H)[:, None] * DH + PERM64[None, :]).reshape(-1)


def _tables64(pos):
    """pos [rows, 2] -> cos/sin [rows, 64] in permuted within-head order;
    sin sign-baked (- for first halves)."""
    rdim = DH // 2
    inv_freq = (
        1.0 / (10000.0 ** (np.arange(0, rdim, 2, dtype=np.float32) / rdim))
    ).astype(np.float32)
    t = pos.astype(np.float32) * np.float32(64.0)  # SCALE / MIN_FREQ
    fx = t[:, 0:1] * inv_freq
    fy = t[:, 1:2] * inv_freq
    f = np.concatenate([fx, fy, fx, fy], axis=1)
    c = np.cos(f)
    s = np.sin(f)
    s[:, :32] *= -1.0
    return c.astype(BF), s.astype(BF)


def _prepare(x, z, x_pos, z_pos, Wq, Wkv, k_gamma, k_beta, v_gamma, v_beta, Wo, bo):
    triv = (
        np.all(np.asarray(k_gamma) == 1.0)
        and np.all(np.asarray(k_beta) == 0.0)
        and np.all(np.asarray(v_gamma) == 1.0)
        and np.all(np.asarray(v_beta) == 0.0)
    )
    assert triv, "non-trivial gamma/beta not supported by this kernel"

    xf = np.asarray(x, dtype=np.float32).reshape(B * N1, DIM)
    zf = np.asarray(z, dtype=np.float32).reshape(B * N1, DIM)
    xpf = np.asarray(x_pos).reshape(B * N1, 2)
    zpf = np.asarray(z_pos).reshape(B * N1, 2)

    wqT = ((np.asarray(Wq).T / np.float32(N1))[:, PERM]).astype(BF)
    wkvT = np.asarray(Wkv).T.astype(np.float32)  # [256, 1024]
    wkv_c = wkvT.reshape(DIM, 16, DH)
    wkvT = (wkv_c - wkv_c.mean(axis=2, keepdims=True)).reshape(DIM, 2 * INNER)
    wkvT = np.concatenate(
        [wkvT[:, :INNER][:, PERM], wkvT[:, INNER:]], axis=1
    ).astype(BF)
    woT = np.ascontiguousarray(np.asarray(Wo).T).astype(BF)  # [512, 256]

    cq, sq_ = _tables64(xpf)
    ck, sk_ = _tables64(zpf)

    nc = build_nc()
    in_maps = []
    for c in range(NCORES):
        lo, hi = c * ROWS, (c + 1) * ROWS
        in_maps.append(
            {
                "xT": np.ascontiguousarray(xf[lo:hi].T).astype(BF),
                "zT": np.ascontiguousarray(zf[lo:hi].T).astype(BF),
                "wq": np.ascontiguousarray(wqT),
                "wkv": np.ascontiguousarray(wkvT),
                "wo": woT,
                "cq": np.ascontiguousarray(cq[lo:hi]),
                "sq": np.ascontiguousarray(sq_[lo:hi]),
                "ck": np.ascontiguousarray(ck[lo:hi]),
                "sk": np.ascontiguousarray(sk_[lo:hi]),
            }
        )
    return nc, in_maps


def _finalize(results, bo):
    ys = [
        np.asarray(results[c]["yT"]).astype(np.float32).T for c in range(NCORES)
    ]  # each [4096, 256]
    y = np.concatenate(ys, axis=0) + np.asarray(bo, dtype=np.float32)
    return y.reshape(B, N1, DIM).astype(np.float32)


def kernel(**inputs):
    nc, in_maps = _prepare(**inputs)
    res = run_bass_kernel_spmd(nc, in_maps, list(range(NCORES)))
    return _finalize(res.results, inputs["bo"])


# revision 4
# speedup vs baseline: 1.0042x; 1.0042x over previous
"""CrossLinearAttention Trainium2 kernel v2 (8 NeuronCores, SPMD).

b=4, n1=n2=8192, dim=256, 8 heads x 64. Flat rows 32768 -> 4096/core.
Each core's shard lies in one batch element (b = c // 2); partial
k^T v (dotsT) AllReduced between core pairs.

v2 vs baseline:
  - bf16 matmul operands everywhere (tolerance 2e-2 >> bf16 error).
  - Host pre-transposes x, z -> xT, zT [256, rows]; projections use
    xT/zT chunks as the stationary operand: no PE transposes of
    activations, no PSUM->SBUF copies on the z side.
  - Within-head feature order permuted to [x1st16, y1st16, x2nd16,
    y2nd16] so rotate-half is a uniform +-32 column shift (single
    strided view); sin tables sign-baked.
  - Rotary add fused into dots PSUM accumulation: dotsT accumulates
    (v2^T kc) + (v2^T ks) over all tiles.
  - dots -> output projection fused: W2 = blkdiag(dotsT) @ Wo^T
    computed once on device after AllReduce; y^T = W2^T-chunks @ qT,
    DMA'd from PSUM to DRAM; host un-transposes and adds bias.
"""

import sys

sys.path.insert(0, "/opt/trn_rl_repo")

from contextlib import ExitStack

import ml_dtypes
import numpy as np

import concourse.bass as bass
import concourse.tile as tile
from concourse import bacc, mybir
from concourse.bass import ts
from concourse.bass_utils import run_bass_kernel_spmd
from concourse.masks import make_identity

F32 = mybir.dt.float32
BF16 = mybir.dt.bfloat16
ALU = mybir.AluOpType
AX = mybir.AxisListType
AF = mybir.ActivationFunctionType

B, N1, DIM = 4, 8192, 256
H, DH = 8, 64
INNER = H * DH  # 512
NCORES = 8
ROWS = (B * N1) // NCORES  # 4096 rows per core
NT = ROWS // 128  # 32 tiles of 128 rows
EPS = 1e-5
BF = ml_dtypes.bfloat16

_nc_cache = {}


def _view(ap, offset, dims):
    """AP view with explicit free dims; keeps the partition dim."""
    return bass.AP(
        tensor=ap.tensor,
        offset=ap.offset + offset,
        ap=[list(ap.ap[0])] + [list(d) for d in dims],
    )


def build_nc():
    if "nc" in _nc_cache:
        return _nc_cache["nc"]
    nc = bacc.Bacc(trn_type="TRN2", num_devices=NCORES, debug=False)

    xT_d = nc.dram_tensor("xT", [DIM, ROWS], BF16, kind="ExternalInput").ap()
    zT_d = nc.dram_tensor("zT", [DIM, ROWS], BF16, kind="ExternalInput").ap()
    wq_d = nc.dram_tensor("wq", [DIM, INNER], BF16, kind="ExternalInput").ap()
    wkv_d = nc.dram_tensor("wkv", [DIM, 2 * INNER], BF16, kind="ExternalInput").ap()
    wo_d = nc.dram_tensor("wo", [INNER, DIM], BF16, kind="ExternalInput").ap()
    cq_d = nc.dram_tensor("cq", [ROWS, DH], BF16, kind="ExternalInput").ap()
    sq_d = nc.dram_tensor("sq", [ROWS, DH], BF16, kind="ExternalInput").ap()
    ck_d = nc.dram_tensor("ck", [ROWS, DH], BF16, kind="ExternalInput").ap()
    sk_d = nc.dram_tensor("sk", [ROWS, DH], BF16, kind="ExternalInput").ap()
    yT_d = nc.dram_tensor("yT", [DIM, ROWS], BF16, kind="ExternalOutput").ap()
    cc_in = nc.dram_tensor("cc_in", [128, INNER], F32)
    cc_out = nc.dram_tensor("cc_out", [128, INNER], F32)

    with tile.TileContext(nc) as tc, ExitStack() as ctx:
        consts = ctx.enter_context(tc.tile_pool(name="consts", bufs=1))

        ident_f = consts.tile([128, 128], F32)
        make_identity(nc, ident_f)
        ident = consts.tile([128, 128], BF16)
        nc.vector.tensor_copy(ident, ident_f)
        eps_t = consts.tile([128, 1], F32)
        nc.vector.memset(eps_t, EPS)

        # phase-A-critical consts first on the sync queue; B-side consts
        # ride other engine queues so they stream in behind A's compute
        zT_sb, xT_sb, wq_t, wkv_t = [], [], [], []
        # first tiles' tables land before the bulk loads so tile 0's
        # vector work starts as soon as its kv matmul completes
        ckE = [consts.tile([128, DH], BF16, tag=f"ckE{t}", name=f"ckE{t}")
               for t in range(4)]
        skE = [consts.tile([128, DH], BF16, tag=f"skE{t}", name=f"skE{t}")
               for t in range(4)]
        for t in range(4):
            nc.sync.dma_start(ckE[t], ck_d[ts(t, 128), :])
            nc.sync.dma_start(skE[t], sk_d[ts(t, 128), :])
        for c in range(2):
            t = consts.tile([128, 2 * INNER], BF16, tag=f"wkv{c}")
            nc.sync.dma_start(t, wkv_d[ts(c, 128), :])
            wkv_t.append(t)
        NCH = 4
        CH = ROWS // NCH
        for c in range(2):
            zT_sb.append(
                [consts.tile([128, CH], BF16, tag=f"zT{c}_{j}", name=f"zT{c}_{j}")
                 for j in range(NCH)]
            )
            xT_sb.append(
                [consts.tile([128, CH], BF16, tag=f"xT{c}_{j}", name=f"xT{c}_{j}")
                 for j in range(NCH)]
            )
        for j in range(NCH):
            for c in range(2):
                nc.sync.dma_start(zT_sb[c][j], zT_d[ts(c, 128), ts(j, CH)])
        for c in range(2):
            t = consts.tile([128, INNER], BF16, tag=f"wq{c}")
            nc.scalar.dma_start(t, wq_d[ts(c, 128), :])
            wq_t.append(t)
        for j in range(NCH):
            for c in range(2):
                nc.scalar.dma_start(xT_sb[c][j], xT_d[ts(c, 128), ts(j, CH)])

        def zchunk(c, t):
            return zT_sb[c][t // (NT // NCH)][:, ts(t % (NT // NCH), 128)]

        def xchunk(c, t):
            return xT_sb[c][t // (NT // NCH)][:, ts(t % (NT // NCH), 128)]
        wo_t = []
        for p in range(4):
            t = consts.tile([128, DIM], BF16, tag=f"wo{p}")
            nc.gpsimd.dma_start(t, wo_d[ts(p, 128), :])
            wo_t.append(t)
        # qT staging: tile t occupies cols [512t, 512t+512), f-chunk f at
        # [512t+128f, 512t+128f+128). yT matmuls read strided views.
        qTb = consts.tile([128, NT * INNER], BF16)

        # ---------------- phase A: z side -> dotsT ----------------
        with tc.tile_pool(name="kv_pp", bufs=2, space="PSUM") as kv_pp, \
             tc.tile_pool(name="dots_pp", bufs=1, space="PSUM") as dots_pp, \
             tc.tile_pool(name="tabsA", bufs=5) as tabsA, \
             tc.tile_pool(name="elemA", bufs=5) as elemA, \
             tc.tile_pool(name="statsA", bufs=6) as statsA:

            dots_ps = []
            for p in range(4):
                dots_ps.append(
                    dots_pp.tile([128, 512], F32, tag=f"dots{p}", name=f"dots{p}")
                )

            for t in range(NT):
                if t < 4:
                    ck_t, sk_t = ckE[t], skE[t]
                else:
                    ck_t = tabsA.tile([128, DH], BF16, tag="ck")
                    nc.sync.dma_start(ck_t, ck_d[ts(t, 128), :])
                    sk_t = tabsA.tile([128, DH], BF16, tag="sk")
                    nc.sync.dma_start(sk_t, sk_d[ts(t, 128), :])

                kv_ps = kv_pp.tile([128, 2 * INNER], F32, tag="kv")
                for c in range(2):
                    nc.tensor.matmul(
                        kv_ps[:, 0:INNER],
                        zchunk(c, t),
                        wkv_t[c][:, 0:INNER],
                        start=(c == 0),
                        stop=(c == 1),
                    )
                for c in range(2):
                    nc.tensor.matmul(
                        kv_ps[:, INNER : 2 * INNER],
                        zchunk(c, t),
                        wkv_t[c][:, INNER : 2 * INNER],
                        start=(c == 0),
                        stop=(c == 1),
                    )

                # single PSUM reader: convert then everything reads SBUF bf16
                kvb_t = elemA.tile([128, 2 * INNER], BF16, tag="kvb")
                nc.scalar.copy(kvb_t, kv_ps)

                sq_t = elemA.tile([128, 2 * INNER], BF16, tag="sq")
                nc.scalar.activation(sq_t[:, 0:INNER], kvb_t[:, 0:INNER], AF.Square)
                nc.vector.tensor_tensor(
                    sq_t[:, INNER : 2 * INNER],
                    kvb_t[:, INNER : 2 * INNER],
                    kvb_t[:, INNER : 2 * INNER],
                    op=ALU.mult,
                )
                red_t = statsA.tile([128, 16], F32, tag="red")
                nc.vector.reduce_sum(
                    red_t, _view(sq_t, 0, [[64, 16], [1, 64]]), axis=AX.X
                )
                std_t = statsA.tile([128, 16], F32, tag="std")
                nc.scalar.activation(
                    std_t, red_t, AF.Sqrt, scale=1.0 / DH, bias=eps_t[:, 0:1]
                )
                rstd_t = statsA.tile([128, 16], F32, tag="rstd")
                nc.vector.reciprocal(rstd_t, std_t)
                rkv_t = statsA.tile([128, 8], F32, tag="rkv")
                nc.vector.tensor_tensor(
                    rkv_t, rstd_t[:, 0:8], rstd_t[:, 8:16], op=ALU.mult
                )

                kc_t = elemA.tile([128, INNER], BF16, tag="kc")
                nc.vector.tensor_tensor(
                    kc_t,
                    kvb_t[:, 0:INNER],
                    _view(ck_t, 0, [[0, 8], [1, 64]]),
                    op=ALU.mult,
                )
                ks_t = elemA.tile([128, INNER], BF16, tag="ks")
                nc.gpsimd.tensor_tensor(
                    _view(ks_t, 0, [[64, 8], [32, 2], [1, 32]]),
                    _view(kvb_t, 32, [[64, 8], [-32, 2], [1, 32]]),
                    _view(sk_t, 0, [[0, 8], [32, 2], [1, 32]]),
                    op=ALU.mult,
                )
                v2_t = elemA.tile([128, INNER], BF16, tag="v2")
                nc.vector.tensor_tensor(
                    v2_t[:, 0:256],
                    kvb_t[:, INNER : INNER + 256],
                    _view(rkv_t, 0, [[1, 4], [0, 64]]),
                    op=ALU.mult,
                )
                nc.gpsimd.tensor_tensor(
                    v2_t[:, 256:512],
                    kvb_t[:, INNER + 256 : 2 * INNER],
                    _view(rkv_t, 4, [[1, 4], [0, 64]]),
                    op=ALU.mult,
                )

                for p in range(4):
                    nc.tensor.matmul(
                        dots_ps[p][:, 0:128],
                        v2_t[:, ts(p, 128)],
                        kc_t[:, ts(p, 128)],
                        start=(t == 0),
                        stop=False,
                    )
                    nc.tensor.matmul(
                        dots_ps[p][:, 0:128],
                        v2_t[:, ts(p, 128)],
                        ks_t[:, ts(p, 128)],
                        start=False,
                        stop=(t == NT - 1),
                    )

            dotsT_sb = consts.tile([128, INNER], F32)
            for p in range(4):
                nc.vector.tensor_copy(dotsT_sb[:, ts(p, 128)], dots_ps[p][:, 0:128])

        # collective rides the gpsimd queue (idle during phase B) so its
        # DMAs don't block phase B's table loads on the sync queue
        nc.gpsimd.dma_start(cc_in.ap(), dotsT_sb)
        nc.gpsimd.collective_compute(
            "AllReduce",
            ALU.add,
            replica_groups=[[0, 1], [2, 3], [4, 5], [6, 7]],
            ins=[cc_in.ap()],
            outs=[cc_out.ap()],
        )
        dots_rd = consts.tile([128, INNER], F32)
        nc.gpsimd.dma_start(dots_rd, cc_out.ap())

        # ------- phase B: q side (overlaps AllReduce) + interleaved yT -------
        with tc.tile_pool(name="q_pp", bufs=2, space="PSUM") as q_pp, \
             tc.tile_pool(name="qt_pp", bufs=2, space="PSUM") as qt_pp, \
             tc.tile_pool(name="w2_pp", bufs=2, space="PSUM") as w2_pp, \
             tc.tile_pool(name="y_pp", bufs=2, space="PSUM") as y_pp, \
             tc.tile_pool(name="tabsB", bufs=5) as tabsB, \
             tc.tile_pool(name="elemB", bufs=5) as elemB, \
             tc.tile_pool(name="yout", bufs=4) as yout:

            W2b = [
                consts.tile([128, DIM], BF16, tag=f"W2b{p}", name=f"W2b{p}")
                for p in range(4)
            ]

            def b_tile(t):
                cq_t = tabsB.tile([128, DH], BF16, tag="cq")
                nc.sync.dma_start(cq_t, cq_d[ts(t, 128), :])
                sq2_t = tabsB.tile([128, DH], BF16, tag="sq2")
                nc.sync.dma_start(sq2_t, sq_d[ts(t, 128), :])

                q_ps = q_pp.tile([128, INNER], F32, tag="q")
                for c in range(2):
                    nc.tensor.matmul(
                        q_ps,
                        xchunk(c, t),
                        wq_t[c],
                        start=(c == 0),
                        stop=(c == 1),
                    )
                qb_t = elemB.tile([128, INNER], BF16, tag="qb")
                nc.scalar.copy(qb_t, q_ps)
                qc_t = elemB.tile([128, INNER], BF16, tag="qc")
                nc.vector.tensor_tensor(
                    qc_t, qb_t, _view(cq_t, 0, [[0, 8], [1, 64]]), op=ALU.mult
                )
                qs_t = elemB.tile([128, INNER], BF16, tag="qs")
                nc.vector.tensor_tensor(
                    _view(qs_t, 0, [[64, 8], [32, 2], [1, 32]]),
                    _view(qb_t, 32, [[64, 8], [-32, 2], [1, 32]]),
                    _view(sq2_t, 0, [[0, 8], [32, 2], [1, 32]]),
                    op=ALU.mult,
                )
                qr_t = elemB.tile([128, INNER], BF16, tag="qr")
                nc.vector.tensor_tensor(qr_t, qc_t, qs_t, op=ALU.add)

                qT_ps = qt_pp.tile([128, INNER], BF16, tag="qT")
                for f in range(4):
                    nc.tensor.transpose(
                        qT_ps[:, ts(f, 128)], qr_t[:, ts(f, 128)], ident
                    )
                nc.scalar.copy(qTb[:, ts(t, INNER)], qT_ps)

            def yt_group(r):
                for o in range(2):
                    yT_ps = y_pp.tile([128, 512], F32, tag="yT")
                    for f in range(4):
                        nc.tensor.matmul(
                            yT_ps,
                            W2b[f][:, ts(o, 128)],
                            _view(qTb, r * 4 * INNER + f * 128,
                                  [[INNER, 4], [1, 128]]),
                            start=(f == 0),
                            stop=(f == 3),
                        )
                    yT_sb = yout.tile([128, 512], BF16, tag="ysb")
                    if o == 0:
                        nc.vector.tensor_copy(yT_sb, yT_ps)
                    else:
                        nc.scalar.copy(yT_sb, yT_ps)
                    nc.sync.dma_start(yT_d[ts(o, 128), ts(r, 512)], yT_sb)

            for t in range(24):
                b_tile(t)

            # W2 = blkdiag(dotsT) @ WoT (collective done by now)
            blk = consts.tile([128, INNER], BF16)
            nc.vector.memset(blk, 0.0)
            nc.vector.tensor_copy(
                _view(blk[0:64, 0:1], 0, [[128, 4], [1, 64]]),
                _view(dots_rd[0:64, 0:1], 0, [[128, 4], [1, 64]]),
            )
            nc.vector.tensor_copy(
                _view(blk[64:128, 0:1], 64, [[128, 4], [1, 64]]),
                _view(dots_rd[64:128, 0:1], 64, [[128, 4], [1, 64]]),
            )
            for p in range(4):
                w2_ps = w2_pp.tile([128, 512], F32, tag="w2")
                nc.tensor.matmul(
                    w2_ps[:, 0:DIM], blk[:, ts(p, 128)], wo_t[p],
                    start=True, stop=True,
                )
                nc.scalar.copy(W2b[p], w2_ps[:, 0:DIM])

            for r in range(5):
                yt_group(r)
            for t in range(24, 28):
                b_tile(t)
            yt_group(5)
            for t in range(28, 32):
                b_tile(t)
            yt_group(6)
            yt_group(7)

    nc.compile()
    _nc_cache["nc"] = nc
    return nc


PERM64 = np.concatenate(
    [np.arange(0, 16), np.arange(32, 48), np.arange(16, 32), np.arange(48, 64)]
)
PERM = (np.arange(H)[:, None] * DH + PERM64[None, :]).reshape(-1)


def _tables64(pos):
    """pos [rows, 2] -> cos/sin [rows, 64] in permuted within-head order;
    sin sign-baked (- for first halves)."""
    rdim = DH // 2
    inv_freq = (
        1.0 / (10000.0 ** (np.arange(0, rdim, 2, dtype=np.float32) / rdim))
    ).astype(np.float32)
    t = pos.astype(np.float32) * np.float32(64.0)  # SCALE / MIN_FREQ
    fx = t[:, 0:1] * inv_freq
    fy = t[:, 1:2] * inv_freq
    f = np.concatenate([fx, fy, fx, fy], axis=1)
    c = np.cos(f)
    s = np.sin(f)
    s[:, :32] *= -1.0
    return c.astype(BF), s.astype(BF)


def _prepare(x, z, x_pos, z_pos, Wq, Wkv, k_gamma, k_beta, v_gamma, v_beta, Wo, bo):
    triv = (
        np.all(np.asarray(k_gamma) == 1.0)
        and np.all(np.asarray(k_beta) == 0.0)
        and np.all(np.asarray(v_gamma) == 1.0)
        and np.all(np.asarray(v_beta) == 0.0)
    )
    assert triv, "non-trivial gamma/beta not supported by this kernel"

    xf = np.asarray(x, dtype=np.float32).reshape(B * N1, DIM)
    zf = np.asarray(z, dtype=np.float32).reshape(B * N1, DIM)
    xpf = np.asarray(x_pos).reshape(B * N1, 2)
    zpf = np.asarray(z_pos).reshape(B * N1, 2)

    wqT = ((np.asarray(Wq).T / np.float32(N1))[:, PERM]).astype(BF)
    wkvT = np.asarray(Wkv).T.astype(np.float32)  # [256, 1024]
    wkv_c = wkvT.reshape(DIM, 16, DH)
    wkvT = (wkv_c - wkv_c.mean(axis=2, keepdims=True)).reshape(DIM, 2 * INNER)
    wkvT = np.concatenate(
        [wkvT[:, :INNER][:, PERM], wkvT[:, INNER:]], axis=1
    ).astype(BF)
    woT = np.ascontiguousarray(np.asarray(Wo).T).astype(BF)  # [512, 256]

    cq, sq_ = _tables64(xpf)
    ck, sk_ = _tables64(zpf)

    nc = build_nc()
    in_maps = []
    for c in range(NCORES):
        lo, hi = c * ROWS, (c + 1) * ROWS
        in_maps.append(
            {
                "xT": np.ascontiguousarray(xf[lo:hi].T).astype(BF),
                "zT": np.ascontiguousarray(zf[lo:hi].T).astype(BF),
                "wq": np.ascontiguousarray(wqT),
                "wkv": np.ascontiguousarray(wkvT),
                "wo": woT,
                "cq": np.ascontiguousarray(cq[lo:hi]),
                "sq": np.ascontiguousarray(sq_[lo:hi]),
                "ck": np.ascontiguousarray(ck[lo:hi]),
                "sk": np.ascontiguousarray(sk_[lo:hi]),
            }
        )
    return nc, in_maps


def _finalize(results, bo):
    ys = [
        np.asarray(results[c]["yT"]).astype(np.float32).T for c in range(NCORES)
    ]  # each [4096, 256]
    y = np.concatenate(ys, axis=0) + np.asarray(bo, dtype=np.float32)
    return y.reshape(B, N1, DIM).astype(np.float32)


def kernel(**inputs):
    nc, in_maps = _prepare(**inputs)
    res = run_bass_kernel_spmd(nc, in_maps, list(range(NCORES)))
    return _finalize(res.results, inputs["bo"])


# revision 5
# speedup vs baseline: 1.0570x; 1.0525x over previous
"""CrossLinearAttention Trainium2 kernel v2 (8 NeuronCores, SPMD).

b=4, n1=n2=8192, dim=256, 8 heads x 64. Flat rows 32768 -> 4096/core.
Each core's shard lies in one batch element (b = c // 2); partial
k^T v (dotsT) AllReduced between core pairs.

v2 vs baseline:
  - bf16 matmul operands everywhere (tolerance 2e-2 >> bf16 error).
  - Host pre-transposes x, z -> xT, zT [256, rows]; projections use
    xT/zT chunks as the stationary operand: no PE transposes of
    activations, no PSUM->SBUF copies on the z side.
  - Within-head feature order permuted to [x1st16, y1st16, x2nd16,
    y2nd16] so rotate-half is a uniform +-32 column shift (single
    strided view); sin tables sign-baked.
  - Rotary add fused into dots PSUM accumulation: dotsT accumulates
    (v2^T kc) + (v2^T ks) over all tiles.
  - dots -> output projection fused: W2 = blkdiag(dotsT) @ Wo^T
    computed once on device after AllReduce; y^T = W2^T-chunks @ qT,
    DMA'd from PSUM to DRAM; host un-transposes and adds bias.
"""

import sys

sys.path.insert(0, "/opt/trn_rl_repo")

from contextlib import ExitStack

import ml_dtypes
import numpy as np

import concourse.bass as bass
import concourse.tile as tile
from concourse import bacc, mybir
from concourse.bass import ts
from concourse.bass_utils import run_bass_kernel_spmd
from concourse.masks import make_identity

F32 = mybir.dt.float32
BF16 = mybir.dt.bfloat16
ALU = mybir.AluOpType
AX = mybir.AxisListType
AF = mybir.ActivationFunctionType

B, N1, DIM = 4, 8192, 256
H, DH = 8, 64
INNER = H * DH  # 512
NCORES = 8
ROWS = (B * N1) // NCORES  # 4096 rows per core
NT = ROWS // 128  # 32 tiles of 128 rows
EPS = 1e-5
BF = ml_dtypes.bfloat16

_nc_cache = {}


def _view(ap, offset, dims):
    """AP view with explicit free dims; keeps the partition dim."""
    return bass.AP(
        tensor=ap.tensor,
        offset=ap.offset + offset,
        ap=[list(ap.ap[0])] + [list(d) for d in dims],
    )


def build_nc():
    if "nc" in _nc_cache:
        return _nc_cache["nc"]
    nc = bacc.Bacc(trn_type="TRN2", num_devices=NCORES, debug=False)

    xT_d = nc.dram_tensor("xT", [DIM, ROWS], BF16, kind="ExternalInput").ap()
    zT_d = nc.dram_tensor("zT", [DIM, ROWS], BF16, kind="ExternalInput").ap()
    wq_d = nc.dram_tensor("wq", [DIM, INNER], BF16, kind="ExternalInput").ap()
    wkv_d = nc.dram_tensor("wkv", [DIM, 2 * INNER], BF16, kind="ExternalInput").ap()
    wo_d = nc.dram_tensor("wo", [INNER, DIM], BF16, kind="ExternalInput").ap()
    cq_d = nc.dram_tensor("cq", [ROWS, DH], BF16, kind="ExternalInput").ap()
    sq_d = nc.dram_tensor("sq", [ROWS, DH], BF16, kind="ExternalInput").ap()
    ck_d = nc.dram_tensor("ck", [ROWS, DH], BF16, kind="ExternalInput").ap()
    sk_d = nc.dram_tensor("sk", [ROWS, DH], BF16, kind="ExternalInput").ap()
    yT_d = nc.dram_tensor("yT", [DIM, ROWS], BF16, kind="ExternalOutput").ap()
    cc_in = nc.dram_tensor("cc_in", [128, INNER], F32)
    cc_out = nc.dram_tensor("cc_out", [128, INNER], F32)

    with tile.TileContext(nc) as tc, ExitStack() as ctx:
        consts = ctx.enter_context(tc.tile_pool(name="consts", bufs=1))

        ident_f = consts.tile([128, 128], F32)
        make_identity(nc, ident_f)
        ident = consts.tile([128, 128], BF16)
        nc.vector.tensor_copy(ident, ident_f)
        eps_t = consts.tile([128, 1], F32)
        nc.vector.memset(eps_t, EPS)

        # phase-A-critical consts first on the sync queue; B-side consts
        # ride other engine queues so they stream in behind A's compute
        zT_sb, xT_sb, wq_t, wkv_t = [], [], [], []
        # first tiles' tables land before the bulk loads so tile 0's
        # vector work starts as soon as its kv matmul completes
        ckE = [consts.tile([128, DH], BF16, tag=f"ckE{t}", name=f"ckE{t}")
               for t in range(4)]
        skE = [consts.tile([128, DH], BF16, tag=f"skE{t}", name=f"skE{t}")
               for t in range(4)]
        for t in range(4):
            nc.sync.dma_start(ckE[t], ck_d[ts(t, 128), :])
            nc.sync.dma_start(skE[t], sk_d[ts(t, 128), :])
        for c in range(2):
            t = consts.tile([128, 2 * INNER], BF16, tag=f"wkv{c}")
            nc.sync.dma_start(t, wkv_d[ts(c, 128), :])
            wkv_t.append(t)
        NCH = 4
        CH = ROWS // NCH
        for c in range(2):
            zT_sb.append(
                [consts.tile([128, CH], BF16, tag=f"zT{c}_{j}", name=f"zT{c}_{j}")
                 for j in range(NCH)]
            )
            xT_sb.append(
                [consts.tile([128, CH], BF16, tag=f"xT{c}_{j}", name=f"xT{c}_{j}")
                 for j in range(NCH)]
            )
        for j in range(NCH):
            for c in range(2):
                nc.sync.dma_start(zT_sb[c][j], zT_d[ts(c, 128), ts(j, CH)])
        for c in range(2):
            t = consts.tile([128, INNER], BF16, tag=f"wq{c}")
            nc.scalar.dma_start(t, wq_d[ts(c, 128), :])
            wq_t.append(t)
        for j in range(NCH):
            for c in range(2):
                nc.scalar.dma_start(xT_sb[c][j], xT_d[ts(c, 128), ts(j, CH)])

        def zchunk(c, t):
            return zT_sb[c][t // (NT // NCH)][:, ts(t % (NT // NCH), 128)]

        def xchunk(c, t):
            return xT_sb[c][t // (NT // NCH)][:, ts(t % (NT // NCH), 128)]
        wo_t = []
        for p in range(4):
            t = consts.tile([128, DIM], BF16, tag=f"wo{p}")
            nc.gpsimd.dma_start(t, wo_d[ts(p, 128), :])
            wo_t.append(t)
        # qT staging: tile t occupies cols [512t, 512t+512), f-chunk f at
        # [512t+128f, 512t+128f+128). yT matmuls read strided views.
        qTb = consts.tile([128, NT * INNER], BF16)

        # ---------------- phase A: z side -> dotsT ----------------
        with tc.tile_pool(name="kv_pp", bufs=2, space="PSUM") as kv_pp, \
             tc.tile_pool(name="dots_pp", bufs=1, space="PSUM") as dots_pp, \
             tc.tile_pool(name="tabsA", bufs=8) as tabsA, \
             tc.tile_pool(name="elemA", bufs=8) as elemA, \
             tc.tile_pool(name="statsA", bufs=10) as statsA:

            dots_ps = []
            for p in range(4):
                dots_ps.append(
                    dots_pp.tile([128, 512], F32, tag=f"dots{p}", name=f"dots{p}")
                )

            for t in range(NT):
                if t < 4:
                    ck_t, sk_t = ckE[t], skE[t]
                else:
                    ck_t = tabsA.tile([128, DH], BF16, tag="ck")
                    nc.sync.dma_start(ck_t, ck_d[ts(t, 128), :])
                    sk_t = tabsA.tile([128, DH], BF16, tag="sk")
                    nc.sync.dma_start(sk_t, sk_d[ts(t, 128), :])

                kv_ps = kv_pp.tile([128, 2 * INNER], F32, tag="kv")
                for c in range(2):
                    nc.tensor.matmul(
                        kv_ps[:, 0:INNER],
                        zchunk(c, t),
                        wkv_t[c][:, 0:INNER],
                        start=(c == 0),
                        stop=(c == 1),
                    )
                for c in range(2):
                    nc.tensor.matmul(
                        kv_ps[:, INNER : 2 * INNER],
                        zchunk(c, t),
                        wkv_t[c][:, INNER : 2 * INNER],
                        start=(c == 0),
                        stop=(c == 1),
                    )

                # single PSUM reader: convert then everything reads SBUF bf16
                kvb_t = elemA.tile([128, 2 * INNER], BF16, tag="kvb")
                nc.scalar.copy(kvb_t, kv_ps)

                sq_t = elemA.tile([128, 2 * INNER], BF16, tag="sq")
                nc.scalar.activation(sq_t[:, 0:INNER], kvb_t[:, 0:INNER], AF.Square)
                nc.vector.tensor_tensor(
                    sq_t[:, INNER : 2 * INNER],
                    kvb_t[:, INNER : 2 * INNER],
                    kvb_t[:, INNER : 2 * INNER],
                    op=ALU.mult,
                )
                red_t = statsA.tile([128, 16], F32, tag="red")
                nc.vector.reduce_sum(
                    red_t, _view(sq_t, 0, [[64, 16], [1, 64]]), axis=AX.X
                )
                std_t = statsA.tile([128, 16], F32, tag="std")
                nc.scalar.activation(
                    std_t, red_t, AF.Sqrt, scale=1.0 / DH, bias=eps_t[:, 0:1]
                )
                rstd_t = statsA.tile([128, 16], F32, tag="rstd")
                nc.vector.reciprocal(rstd_t, std_t)
                rkv_t = statsA.tile([128, 8], F32, tag="rkv")
                nc.vector.tensor_tensor(
                    rkv_t, rstd_t[:, 0:8], rstd_t[:, 8:16], op=ALU.mult
                )

                kc_t = elemA.tile([128, INNER], BF16, tag="kc")
                nc.vector.tensor_tensor(
                    kc_t,
                    kvb_t[:, 0:INNER],
                    _view(ck_t, 0, [[0, 8], [1, 64]]),
                    op=ALU.mult,
                )
                ks_t = elemA.tile([128, INNER], BF16, tag="ks")
                nc.gpsimd.tensor_tensor(
                    _view(ks_t, 0, [[64, 8], [32, 2], [1, 32]]),
                    _view(kvb_t, 32, [[64, 8], [-32, 2], [1, 32]]),
                    _view(sk_t, 0, [[0, 8], [32, 2], [1, 32]]),
                    op=ALU.mult,
                )
                v2_t = elemA.tile([128, INNER], BF16, tag="v2")
                nc.vector.tensor_tensor(
                    v2_t[:, 0:256],
                    kvb_t[:, INNER : INNER + 256],
                    _view(rkv_t, 0, [[1, 4], [0, 64]]),
                    op=ALU.mult,
                )
                nc.gpsimd.tensor_tensor(
                    v2_t[:, 256:512],
                    kvb_t[:, INNER + 256 : 2 * INNER],
                    _view(rkv_t, 4, [[1, 4], [0, 64]]),
                    op=ALU.mult,
                )

                for p in range(4):
                    nc.tensor.matmul(
                        dots_ps[p][:, 0:128],
                        v2_t[:, ts(p, 128)],
                        kc_t[:, ts(p, 128)],
                        start=(t == 0),
                        stop=False,
                    )
                    nc.tensor.matmul(
                        dots_ps[p][:, 0:128],
                        v2_t[:, ts(p, 128)],
                        ks_t[:, ts(p, 128)],
                        start=False,
                        stop=(t == NT - 1),
                    )

            dotsT_sb = consts.tile([128, INNER], F32)
            for p in range(4):
                nc.vector.tensor_copy(dotsT_sb[:, ts(p, 128)], dots_ps[p][:, 0:128])

        # collective rides the gpsimd queue (idle during phase B) so its
        # DMAs don't block phase B's table loads on the sync queue
        nc.gpsimd.dma_start(cc_in.ap(), dotsT_sb)
        nc.gpsimd.collective_compute(
            "AllReduce",
            ALU.add,
            replica_groups=[[0, 1], [2, 3], [4, 5], [6, 7]],
            ins=[cc_in.ap()],
            outs=[cc_out.ap()],
        )
        dots_rd = consts.tile([128, INNER], F32)
        nc.gpsimd.dma_start(dots_rd, cc_out.ap())

        # ------- phase B: q side (overlaps AllReduce) + interleaved yT -------
        with tc.tile_pool(name="q_pp", bufs=2, space="PSUM") as q_pp, \
             tc.tile_pool(name="qt_pp", bufs=2, space="PSUM") as qt_pp, \
             tc.tile_pool(name="w2_pp", bufs=2, space="PSUM") as w2_pp, \
             tc.tile_pool(name="y_pp", bufs=2, space="PSUM") as y_pp, \
             tc.tile_pool(name="tabsB", bufs=8) as tabsB, \
             tc.tile_pool(name="elemB", bufs=8) as elemB, \
             tc.tile_pool(name="yout", bufs=6) as yout:

            W2b = [
                consts.tile([128, DIM], BF16, tag=f"W2b{p}", name=f"W2b{p}")
                for p in range(4)
            ]

            def b_tile(t):
                cq_t = tabsB.tile([128, DH], BF16, tag="cq")
                nc.sync.dma_start(cq_t, cq_d[ts(t, 128), :])
                sq2_t = tabsB.tile([128, DH], BF16, tag="sq2")
                nc.sync.dma_start(sq2_t, sq_d[ts(t, 128), :])

                q_ps = q_pp.tile([128, INNER], F32, tag="q")
                for c in range(2):
                    nc.tensor.matmul(
                        q_ps,
                        xchunk(c, t),
                        wq_t[c],
                        start=(c == 0),
                        stop=(c == 1),
                    )
                qb_t = elemB.tile([128, INNER], BF16, tag="qb")
                nc.scalar.copy(qb_t, q_ps)
                qc_t = elemB.tile([128, INNER], BF16, tag="qc")
                nc.vector.tensor_tensor(
                    qc_t, qb_t, _view(cq_t, 0, [[0, 8], [1, 64]]), op=ALU.mult
                )
                qs_t = elemB.tile([128, INNER], BF16, tag="qs")
                nc.vector.tensor_tensor(
                    _view(qs_t, 0, [[64, 8], [32, 2], [1, 32]]),
                    _view(qb_t, 32, [[64, 8], [-32, 2], [1, 32]]),
                    _view(sq2_t, 0, [[0, 8], [32, 2], [1, 32]]),
                    op=ALU.mult,
                )
                qr_t = elemB.tile([128, INNER], BF16, tag="qr")
                nc.vector.tensor_tensor(qr_t, qc_t, qs_t, op=ALU.add)

                qT_ps = qt_pp.tile([128, INNER], BF16, tag="qT")
                for f in range(4):
                    nc.tensor.transpose(
                        qT_ps[:, ts(f, 128)], qr_t[:, ts(f, 128)], ident
                    )
                nc.scalar.copy(qTb[:, ts(t, INNER)], qT_ps)

            def yt_group(r):
                for o in range(2):
                    yT_ps = y_pp.tile([128, 512], F32, tag="yT")
                    for f in range(4):
                        nc.tensor.matmul(
                            yT_ps,
                            W2b[f][:, ts(o, 128)],
                            _view(qTb, r * 4 * INNER + f * 128,
                                  [[INNER, 4], [1, 128]]),
                            start=(f == 0),
                            stop=(f == 3),
                        )
                    yT_sb = yout.tile([128, 512], BF16, tag="ysb")
                    if o == 0:
                        nc.vector.tensor_copy(yT_sb, yT_ps)
                    else:
                        nc.scalar.copy(yT_sb, yT_ps)
                    nc.sync.dma_start(yT_d[ts(o, 128), ts(r, 512)], yT_sb)

            for t in range(24):
                b_tile(t)

            # W2 = blkdiag(dotsT) @ WoT (collective done by now)
            blk = consts.tile([128, INNER], BF16)
            nc.vector.memset(blk, 0.0)
            nc.vector.tensor_copy(
                _view(blk[0:64, 0:1], 0, [[128, 4], [1, 64]]),
                _view(dots_rd[0:64, 0:1], 0, [[128, 4], [1, 64]]),
            )
            nc.vector.tensor_copy(
                _view(blk[64:128, 0:1], 64, [[128, 4], [1, 64]]),
                _view(dots_rd[64:128, 0:1], 64, [[128, 4], [1, 64]]),
            )
            for p in range(4):
                w2_ps = w2_pp.tile([128, 512], F32, tag="w2")
                nc.tensor.matmul(
                    w2_ps[:, 0:DIM], blk[:, ts(p, 128)], wo_t[p],
                    start=True, stop=True,
                )
                nc.scalar.copy(W2b[p], w2_ps[:, 0:DIM])

            for r in range(5):
                yt_group(r)
            for t in range(24, 28):
                b_tile(t)
            yt_group(5)
            for t in range(28, 32):
                b_tile(t)
            yt_group(6)
            yt_group(7)

    nc.compile()
    _nc_cache["nc"] = nc
    return nc


PERM64 = np.concatenate(
    [np.arange(0, 16), np.arange(32, 48), np.arange(16, 32), np.arange(48, 64)]
)
PERM = (np.arange(H)[:, None] * DH + PERM64[None, :]).reshape(-1)


def _tables64(pos):
    """pos [rows, 2] -> cos/sin [rows, 64] in permuted within-head order;
    sin sign-baked (- for first halves)."""
    rdim = DH // 2
    inv_freq = (
        1.0 / (10000.0 ** (np.arange(0, rdim, 2, dtype=np.float32) / rdim))
    ).astype(np.float32)
    t = pos.astype(np.float32) * np.float32(64.0)  # SCALE / MIN_FREQ
    fx = t[:, 0:1] * inv_freq
    fy = t[:, 1:2] * inv_freq
    f = np.concatenate([fx, fy, fx, fy], axis=1)
    c = np.cos(f)
    s = np.sin(f)
    s[:, :32] *= -1.0
    return c.astype(BF), s.astype(BF)


def _prepare(x, z, x_pos, z_pos, Wq, Wkv, k_gamma, k_beta, v_gamma, v_beta, Wo, bo):
    triv = (
        np.all(np.asarray(k_gamma) == 1.0)
        and np.all(np.asarray(k_beta) == 0.0)
        and np.all(np.asarray(v_gamma) == 1.0)
        and np.all(np.asarray(v_beta) == 0.0)
    )
    assert triv, "non-trivial gamma/beta not supported by this kernel"

    xf = np.asarray(x, dtype=np.float32).reshape(B * N1, DIM)
    zf = np.asarray(z, dtype=np.float32).reshape(B * N1, DIM)
    xpf = np.asarray(x_pos).reshape(B * N1, 2)
    zpf = np.asarray(z_pos).reshape(B * N1, 2)

    wqT = ((np.asarray(Wq).T / np.float32(N1))[:, PERM]).astype(BF)
    wkvT = np.asarray(Wkv).T.astype(np.float32)  # [256, 1024]
    wkv_c = wkvT.reshape(DIM, 16, DH)
    wkvT = (wkv_c - wkv_c.mean(axis=2, keepdims=True)).reshape(DIM, 2 * INNER)
    wkvT = np.concatenate(
        [wkvT[:, :INNER][:, PERM], wkvT[:, INNER:]], axis=1
    ).astype(BF)
    woT = np.ascontiguousarray(np.asarray(Wo).T).astype(BF)  # [512, 256]

    cq, sq_ = _tables64(xpf)
    ck, sk_ = _tables64(zpf)

    nc = build_nc()
    in_maps = []
    for c in range(NCORES):
        lo, hi = c * ROWS, (c + 1) * ROWS
        in_maps.append(
            {
                "xT": np.ascontiguousarray(xf[lo:hi].T).astype(BF),
                "zT": np.ascontiguousarray(zf[lo:hi].T).astype(BF),
                "wq": np.ascontiguousarray(wqT),
                "wkv": np.ascontiguousarray(wkvT),
                "wo": woT,
                "cq": np.ascontiguousarray(cq[lo:hi]),
                "sq": np.ascontiguousarray(sq_[lo:hi]),
                "ck": np.ascontiguousarray(ck[lo:hi]),
                "sk": np.ascontiguousarray(sk_[lo:hi]),
            }
        )
    return nc, in_maps


def _finalize(results, bo):
    ys = [
        np.asarray(results[c]["yT"]).astype(np.float32).T for c in range(NCORES)
    ]  # each [4096, 256]
    y = np.concatenate(ys, axis=0) + np.asarray(bo, dtype=np.float32)
    return y.reshape(B, N1, DIM).astype(np.float32)


def kernel(**inputs):
    nc, in_maps = _prepare(**inputs)
    res = run_bass_kernel_spmd(nc, in_maps, list(range(NCORES)))
    return _finalize(res.results, inputs["bo"])


# revision 6
# speedup vs baseline: 1.0677x; 1.0102x over previous
"""CrossLinearAttention Trainium2 kernel v2 (8 NeuronCores, SPMD).

b=4, n1=n2=8192, dim=256, 8 heads x 64. Flat rows 32768 -> 4096/core.
Each core's shard lies in one batch element (b = c // 2); partial
k^T v (dotsT) AllReduced between core pairs.

v2 vs baseline:
  - bf16 matmul operands everywhere (tolerance 2e-2 >> bf16 error).
  - Host pre-transposes x, z -> xT, zT [256, rows]; projections use
    xT/zT chunks as the stationary operand: no PE transposes of
    activations, no PSUM->SBUF copies on the z side.
  - Within-head feature order permuted to [x1st16, y1st16, x2nd16,
    y2nd16] so rotate-half is a uniform +-32 column shift (single
    strided view); sin tables sign-baked.
  - Rotary add fused into dots PSUM accumulation: dotsT accumulates
    (v2^T kc) + (v2^T ks) over all tiles.
  - dots -> output projection fused: W2 = blkdiag(dotsT) @ Wo^T
    computed once on device after AllReduce; y^T = W2^T-chunks @ qT,
    DMA'd from PSUM to DRAM; host un-transposes and adds bias.
"""

import sys

sys.path.insert(0, "/opt/trn_rl_repo")

from contextlib import ExitStack

import ml_dtypes
import numpy as np

import concourse.bass as bass
import concourse.tile as tile
from concourse import bacc, mybir
from concourse.bass import ts
from concourse.bass_utils import run_bass_kernel_spmd
from concourse.masks import make_identity

F32 = mybir.dt.float32
BF16 = mybir.dt.bfloat16
ALU = mybir.AluOpType
AX = mybir.AxisListType
AF = mybir.ActivationFunctionType

B, N1, DIM = 4, 8192, 256
H, DH = 8, 64
INNER = H * DH  # 512
NCORES = 8
ROWS = (B * N1) // NCORES  # 4096 rows per core
NT = ROWS // 128  # 32 tiles of 128 rows
EPS = 1e-5
BF = ml_dtypes.bfloat16

_nc_cache = {}


def _view(ap, offset, dims):
    """AP view with explicit free dims; keeps the partition dim."""
    return bass.AP(
        tensor=ap.tensor,
        offset=ap.offset + offset,
        ap=[list(ap.ap[0])] + [list(d) for d in dims],
    )


def build_nc():
    if "nc" in _nc_cache:
        return _nc_cache["nc"]
    nc = bacc.Bacc(trn_type="TRN2", num_devices=NCORES, debug=False)

    xT_d = nc.dram_tensor("xT", [DIM, ROWS], BF16, kind="ExternalInput").ap()
    zT_d = nc.dram_tensor("zT", [DIM, ROWS], BF16, kind="ExternalInput").ap()
    wq_d = nc.dram_tensor("wq", [DIM, INNER], BF16, kind="ExternalInput").ap()
    wkv_d = nc.dram_tensor("wkv", [DIM, 2 * INNER], BF16, kind="ExternalInput").ap()
    wo_d = nc.dram_tensor("wo", [INNER, DIM], BF16, kind="ExternalInput").ap()
    cq_d = nc.dram_tensor("cq", [ROWS, DH], BF16, kind="ExternalInput").ap()
    sq_d = nc.dram_tensor("sq", [ROWS, DH], BF16, kind="ExternalInput").ap()
    ck_d = nc.dram_tensor("ck", [ROWS, DH], BF16, kind="ExternalInput").ap()
    sk_d = nc.dram_tensor("sk", [ROWS, DH], BF16, kind="ExternalInput").ap()
    yT_d = nc.dram_tensor("yT", [DIM, ROWS], BF16, kind="ExternalOutput").ap()
    cc_in = nc.dram_tensor("cc_in", [128, INNER], F32)
    cc_out = nc.dram_tensor("cc_out", [128, INNER], F32)

    with tile.TileContext(nc) as tc, ExitStack() as ctx:
        consts = ctx.enter_context(tc.tile_pool(name="consts", bufs=1))

        ident_f = consts.tile([128, 128], F32)
        make_identity(nc, ident_f)
        ident = consts.tile([128, 128], BF16)
        nc.vector.tensor_copy(ident, ident_f)
        eps_t = consts.tile([128, 1], F32)
        nc.vector.memset(eps_t, EPS)

        # phase-A-critical consts first on the sync queue; B-side consts
        # ride other engine queues so they stream in behind A's compute
        zT_sb, xT_sb, wq_t, wkv_t = [], [], [], []
        # first tiles' tables land before the bulk loads so tile 0's
        # vector work starts as soon as its kv matmul completes
        ckE = [consts.tile([128, DH], BF16, tag=f"ckE{t}", name=f"ckE{t}")
               for t in range(4)]
        skE = [consts.tile([128, DH], BF16, tag=f"skE{t}", name=f"skE{t}")
               for t in range(4)]
        for t in range(4):
            nc.sync.dma_start(ckE[t], ck_d[ts(t, 128), :])
            nc.sync.dma_start(skE[t], sk_d[ts(t, 128), :])
        for c in range(2):
            t = consts.tile([128, 2 * INNER], BF16, tag=f"wkv{c}")
            nc.sync.dma_start(t, wkv_d[ts(c, 128), :])
            wkv_t.append(t)
        NCH = 4
        CH = ROWS // NCH
        for c in range(2):
            zT_sb.append(
                [consts.tile([128, CH], BF16, tag=f"zT{c}_{j}", name=f"zT{c}_{j}")
                 for j in range(NCH)]
            )
            xT_sb.append(
                [consts.tile([128, CH], BF16, tag=f"xT{c}_{j}", name=f"xT{c}_{j}")
                 for j in range(NCH)]
            )
        for j in range(NCH):
            for c in range(2):
                nc.sync.dma_start(zT_sb[c][j], zT_d[ts(c, 128), ts(j, CH)])
        for c in range(2):
            t = consts.tile([128, INNER], BF16, tag=f"wq{c}")
            nc.scalar.dma_start(t, wq_d[ts(c, 128), :])
            wq_t.append(t)
        for j in range(NCH):
            for c in range(2):
                nc.scalar.dma_start(xT_sb[c][j], xT_d[ts(c, 128), ts(j, CH)])

        def zchunk(c, t):
            return zT_sb[c][t // (NT // NCH)][:, ts(t % (NT // NCH), 128)]

        def xchunk(c, t):
            return xT_sb[c][t // (NT // NCH)][:, ts(t % (NT // NCH), 128)]
        wo_t = []
        for p in range(4):
            t = consts.tile([128, DIM], BF16, tag=f"wo{p}")
            nc.gpsimd.dma_start(t, wo_d[ts(p, 128), :])
            wo_t.append(t)
        # qT staging: tile t occupies cols [512t, 512t+512), f-chunk f at
        # [512t+128f, 512t+128f+128). yT matmuls read strided views.
        qTb = consts.tile([128, NT * INNER], BF16)

        # ---------------- phase A: z side -> dotsT ----------------
        with tc.tile_pool(name="kv_pp", bufs=2, space="PSUM") as kv_pp, \
             tc.tile_pool(name="dots_pp", bufs=1, space="PSUM") as dots_pp, \
             tc.tile_pool(name="tabsA", bufs=8) as tabsA, \
             tc.tile_pool(name="elemA", bufs=8) as elemA, \
             tc.tile_pool(name="statsA", bufs=10) as statsA:

            dots_ps = []
            for p in range(4):
                dots_ps.append(
                    dots_pp.tile([128, 512], F32, tag=f"dots{p}", name=f"dots{p}")
                )

            for t in range(NT):
                if t < 4:
                    ck_t, sk_t = ckE[t], skE[t]
                else:
                    ck_t = tabsA.tile([128, DH], BF16, tag="ck")
                    nc.sync.dma_start(ck_t, ck_d[ts(t, 128), :])
                    sk_t = tabsA.tile([128, DH], BF16, tag="sk")
                    nc.sync.dma_start(sk_t, sk_d[ts(t, 128), :])

                kv_ps = kv_pp.tile([128, 2 * INNER], F32, tag="kv")
                for c in range(2):
                    nc.tensor.matmul(
                        kv_ps[:, 0:INNER],
                        zchunk(c, t),
                        wkv_t[c][:, 0:INNER],
                        start=(c == 0),
                        stop=(c == 1),
                    )
                for c in range(2):
                    nc.tensor.matmul(
                        kv_ps[:, INNER : 2 * INNER],
                        zchunk(c, t),
                        wkv_t[c][:, INNER : 2 * INNER],
                        start=(c == 0),
                        stop=(c == 1),
                    )

                # single PSUM reader: convert then everything reads SBUF bf16
                kvb_t = elemA.tile([128, 2 * INNER], BF16, tag="kvb")
                nc.scalar.copy(kvb_t, kv_ps)

                sq_t = elemA.tile([128, 2 * INNER], BF16, tag="sq")
                nc.scalar.activation(sq_t, kvb_t, AF.Square)
                red_t = statsA.tile([128, 16], F32, tag="red")
                nc.vector.reduce_sum(
                    red_t, _view(sq_t, 0, [[64, 16], [1, 64]]), axis=AX.X
                )
                std_t = statsA.tile([128, 16], F32, tag="std")
                nc.scalar.activation(
                    std_t, red_t, AF.Sqrt, scale=1.0 / DH, bias=eps_t[:, 0:1]
                )
                rstd_t = statsA.tile([128, 16], F32, tag="rstd")
                nc.vector.reciprocal(rstd_t, std_t)
                rkv_t = statsA.tile([128, 8], F32, tag="rkv")
                nc.vector.tensor_tensor(
                    rkv_t, rstd_t[:, 0:8], rstd_t[:, 8:16], op=ALU.mult
                )

                kc_t = elemA.tile([128, INNER], BF16, tag="kc")
                nc.vector.tensor_tensor(
                    kc_t,
                    kvb_t[:, 0:INNER],
                    _view(ck_t, 0, [[0, 8], [1, 64]]),
                    op=ALU.mult,
                )
                ks_t = elemA.tile([128, INNER], BF16, tag="ks")
                nc.gpsimd.tensor_tensor(
                    _view(ks_t, 0, [[64, 8], [32, 2], [1, 32]]),
                    _view(kvb_t, 32, [[64, 8], [-32, 2], [1, 32]]),
                    _view(sk_t, 0, [[0, 8], [32, 2], [1, 32]]),
                    op=ALU.mult,
                )
                v2_t = elemA.tile([128, INNER], BF16, tag="v2")
                nc.vector.tensor_tensor(
                    v2_t[:, 0:256],
                    kvb_t[:, INNER : INNER + 256],
                    _view(rkv_t, 0, [[1, 4], [0, 64]]),
                    op=ALU.mult,
                )
                nc.gpsimd.tensor_tensor(
                    v2_t[:, 256:512],
                    kvb_t[:, INNER + 256 : 2 * INNER],
                    _view(rkv_t, 4, [[1, 4], [0, 64]]),
                    op=ALU.mult,
                )

                for p in range(4):
                    nc.tensor.matmul(
                        dots_ps[p][:, 0:128],
                        v2_t[:, ts(p, 128)],
                        kc_t[:, ts(p, 128)],
                        start=(t == 0),
                        stop=False,
                    )
                    nc.tensor.matmul(
                        dots_ps[p][:, 0:128],
                        v2_t[:, ts(p, 128)],
                        ks_t[:, ts(p, 128)],
                        start=False,
                        stop=(t == NT - 1),
                    )

            dotsT_sb = consts.tile([128, INNER], F32)
            for p in range(4):
                nc.vector.tensor_copy(dotsT_sb[:, ts(p, 128)], dots_ps[p][:, 0:128])

        # collective rides the gpsimd queue (idle during phase B) so its
        # DMAs don't block phase B's table loads on the sync queue
        nc.gpsimd.dma_start(cc_in.ap(), dotsT_sb)
        nc.gpsimd.collective_compute(
            "AllReduce",
            ALU.add,
            replica_groups=[[0, 1], [2, 3], [4, 5], [6, 7]],
            ins=[cc_in.ap()],
            outs=[cc_out.ap()],
        )
        dots_rd = consts.tile([128, INNER], F32)
        nc.gpsimd.dma_start(dots_rd, cc_out.ap())

        # ------- phase B: q side (overlaps AllReduce) + interleaved yT -------
        with tc.tile_pool(name="q_pp", bufs=2, space="PSUM") as q_pp, \
             tc.tile_pool(name="qt_pp", bufs=2, space="PSUM") as qt_pp, \
             tc.tile_pool(name="w2_pp", bufs=2, space="PSUM") as w2_pp, \
             tc.tile_pool(name="y_pp", bufs=2, space="PSUM") as y_pp, \
             tc.tile_pool(name="tabsB", bufs=8) as tabsB, \
             tc.tile_pool(name="elemB", bufs=8) as elemB, \
             tc.tile_pool(name="yout", bufs=6) as yout:

            W2b = [
                consts.tile([128, DIM], BF16, tag=f"W2b{p}", name=f"W2b{p}")
                for p in range(4)
            ]

            def b_tile(t):
                cq_t = tabsB.tile([128, DH], BF16, tag="cq")
                nc.sync.dma_start(cq_t, cq_d[ts(t, 128), :])
                sq2_t = tabsB.tile([128, DH], BF16, tag="sq2")
                nc.sync.dma_start(sq2_t, sq_d[ts(t, 128), :])

                q_ps = q_pp.tile([128, INNER], F32, tag="q")
                for c in range(2):
                    nc.tensor.matmul(
                        q_ps,
                        xchunk(c, t),
                        wq_t[c],
                        start=(c == 0),
                        stop=(c == 1),
                    )
                qb_t = elemB.tile([128, INNER], BF16, tag="qb")
                nc.scalar.copy(qb_t, q_ps)
                qc_t = elemB.tile([128, INNER], BF16, tag="qc")
                nc.vector.tensor_tensor(
                    qc_t, qb_t, _view(cq_t, 0, [[0, 8], [1, 64]]), op=ALU.mult
                )
                qs_t = elemB.tile([128, INNER], BF16, tag="qs")
                nc.vector.tensor_tensor(
                    _view(qs_t, 0, [[64, 8], [32, 2], [1, 32]]),
                    _view(qb_t, 32, [[64, 8], [-32, 2], [1, 32]]),
                    _view(sq2_t, 0, [[0, 8], [32, 2], [1, 32]]),
                    op=ALU.mult,
                )
                qr_t = elemB.tile([128, INNER], BF16, tag="qr")
                nc.vector.tensor_tensor(qr_t, qc_t, qs_t, op=ALU.add)

                qT_ps = qt_pp.tile([128, INNER], BF16, tag="qT")
                for f in range(4):
                    nc.tensor.transpose(
                        qT_ps[:, ts(f, 128)], qr_t[:, ts(f, 128)], ident
                    )
                nc.scalar.copy(qTb[:, ts(t, INNER)], qT_ps)

            def yt_group(r):
                for o in range(2):
                    yT_ps = y_pp.tile([128, 512], F32, tag="yT")
                    for f in range(4):
                        nc.tensor.matmul(
                            yT_ps,
                            W2b[f][:, ts(o, 128)],
                            _view(qTb, r * 4 * INNER + f * 128,
                                  [[INNER, 4], [1, 128]]),
                            start=(f == 0),
                            stop=(f == 3),
                        )
                    yT_sb = yout.tile([128, 512], BF16, tag="ysb")
                    if o == 0:
                        nc.vector.tensor_copy(yT_sb, yT_ps)
                    else:
                        nc.scalar.copy(yT_sb, yT_ps)
                    nc.sync.dma_start(yT_d[ts(o, 128), ts(r, 512)], yT_sb)

            for t in range(24):
                b_tile(t)

            # W2 = blkdiag(dotsT) @ WoT (collective done by now)
            blk = consts.tile([128, INNER], BF16)
            nc.vector.memset(blk, 0.0)
            nc.vector.tensor_copy(
                _view(blk[0:64, 0:1], 0, [[128, 4], [1, 64]]),
                _view(dots_rd[0:64, 0:1], 0, [[128, 4], [1, 64]]),
            )
            nc.vector.tensor_copy(
                _view(blk[64:128, 0:1], 64, [[128, 4], [1, 64]]),
                _view(dots_rd[64:128, 0:1], 64, [[128, 4], [1, 64]]),
            )
            for p in range(4):
                w2_ps = w2_pp.tile([128, 512], F32, tag="w2")
                nc.tensor.matmul(
                    w2_ps[:, 0:DIM], blk[:, ts(p, 128)], wo_t[p],
                    start=True, stop=True,
                )
                nc.scalar.copy(W2b[p], w2_ps[:, 0:DIM])

            for r in range(5):
                yt_group(r)
            for t in range(24, 28):
                b_tile(t)
            yt_group(5)
            for t in range(28, 32):
                b_tile(t)
            yt_group(6)
            yt_group(7)

    nc.compile()
    _nc_cache["nc"] = nc
    return nc


PERM64 = np.concatenate(
    [np.arange(0, 16), np.arange(32, 48), np.arange(16, 32), np.arange(48, 64)]
)
PERM = (np.arange(H)[:, None] * DH + PERM64[None, :]).reshape(-1)


def _tables64(pos):
    """pos [rows, 2] -> cos/sin [rows, 64] in permuted within-head order;
    sin sign-baked (- for first halves)."""
    rdim = DH // 2
    inv_freq = (
        1.0 / (10000.0 ** (np.arange(0, rdim, 2, dtype=np.float32) / rdim))
    ).astype(np.float32)
    t = pos.astype(np.float32) * np.float32(64.0)  # SCALE / MIN_FREQ
    fx = t[:, 0:1] * inv_freq
    fy = t[:, 1:2] * inv_freq
    f = np.concatenate([fx, fy, fx, fy], axis=1)
    c = np.cos(f)
    s = np.sin(f)
    s[:, :32] *= -1.0
    return c.astype(BF), s.astype(BF)


def _prepare(x, z, x_pos, z_pos, Wq, Wkv, k_gamma, k_beta, v_gamma, v_beta, Wo, bo):
    triv = (
        np.all(np.asarray(k_gamma) == 1.0)
        and np.all(np.asarray(k_beta) == 0.0)
        and np.all(np.asarray(v_gamma) == 1.0)
        and np.all(np.asarray(v_beta) == 0.0)
    )
    assert triv, "non-trivial gamma/beta not supported by this kernel"

    xf = np.asarray(x, dtype=np.float32).reshape(B * N1, DIM)
    zf = np.asarray(z, dtype=np.float32).reshape(B * N1, DIM)
    xpf = np.asarray(x_pos).reshape(B * N1, 2)
    zpf = np.asarray(z_pos).reshape(B * N1, 2)

    wqT = ((np.asarray(Wq).T / np.float32(N1))[:, PERM]).astype(BF)
    wkvT = np.asarray(Wkv).T.astype(np.float32)  # [256, 1024]
    wkv_c = wkvT.reshape(DIM, 16, DH)
    wkvT = (wkv_c - wkv_c.mean(axis=2, keepdims=True)).reshape(DIM, 2 * INNER)
    wkvT = np.concatenate(
        [wkvT[:, :INNER][:, PERM], wkvT[:, INNER:]], axis=1
    ).astype(BF)
    woT = np.ascontiguousarray(np.asarray(Wo).T).astype(BF)  # [512, 256]

    cq, sq_ = _tables64(xpf)
    ck, sk_ = _tables64(zpf)

    nc = build_nc()
    in_maps = []
    for c in range(NCORES):
        lo, hi = c * ROWS, (c + 1) * ROWS
        in_maps.append(
            {
                "xT": np.ascontiguousarray(xf[lo:hi].T).astype(BF),
                "zT": np.ascontiguousarray(zf[lo:hi].T).astype(BF),
                "wq": np.ascontiguousarray(wqT),
                "wkv": np.ascontiguousarray(wkvT),
                "wo": woT,
                "cq": np.ascontiguousarray(cq[lo:hi]),
                "sq": np.ascontiguousarray(sq_[lo:hi]),
                "ck": np.ascontiguousarray(ck[lo:hi]),
                "sk": np.ascontiguousarray(sk_[lo:hi]),
            }
        )
    return nc, in_maps


def _finalize(results, bo):
    ys = [
        np.asarray(results[c]["yT"]).astype(np.float32).T for c in range(NCORES)
    ]  # each [4096, 256]
    y = np.concatenate(ys, axis=0) + np.asarray(bo, dtype=np.float32)
    return y.reshape(B, N1, DIM).astype(np.float32)


def kernel(**inputs):
    nc, in_maps = _prepare(**inputs)
    res = run_bass_kernel_spmd(nc, in_maps, list(range(NCORES)))
    return _finalize(res.results, inputs["bo"])


# revision 7
# speedup vs baseline: 1.0712x; 1.0033x over previous
"""CrossLinearAttention Trainium2 kernel v2 (8 NeuronCores, SPMD).

b=4, n1=n2=8192, dim=256, 8 heads x 64. Flat rows 32768 -> 4096/core.
Each core's shard lies in one batch element (b = c // 2); partial
k^T v (dotsT) AllReduced between core pairs.

v2 vs baseline:
  - bf16 matmul operands everywhere (tolerance 2e-2 >> bf16 error).
  - Host pre-transposes x, z -> xT, zT [256, rows]; projections use
    xT/zT chunks as the stationary operand: no PE transposes of
    activations, no PSUM->SBUF copies on the z side.
  - Within-head feature order permuted to [x1st16, y1st16, x2nd16,
    y2nd16] so rotate-half is a uniform +-32 column shift (single
    strided view); sin tables sign-baked.
  - Rotary add fused into dots PSUM accumulation: dotsT accumulates
    (v2^T kc) + (v2^T ks) over all tiles.
  - dots -> output projection fused: W2 = blkdiag(dotsT) @ Wo^T
    computed once on device after AllReduce; y^T = W2^T-chunks @ qT,
    DMA'd from PSUM to DRAM; host un-transposes and adds bias.
"""

import sys

sys.path.insert(0, "/opt/trn_rl_repo")

from contextlib import ExitStack

import ml_dtypes
import numpy as np

import concourse.bass as bass
import concourse.tile as tile
from concourse import bacc, mybir
from concourse.bass import ts
from concourse.bass_utils import run_bass_kernel_spmd
from concourse.masks import make_identity

F32 = mybir.dt.float32
BF16 = mybir.dt.bfloat16
ALU = mybir.AluOpType
AX = mybir.AxisListType
AF = mybir.ActivationFunctionType

B, N1, DIM = 4, 8192, 256
H, DH = 8, 64
INNER = H * DH  # 512
NCORES = 8
ROWS = (B * N1) // NCORES  # 4096 rows per core
NT = ROWS // 128  # 32 tiles of 128 rows
EPS = 1e-5
BF = ml_dtypes.bfloat16

_nc_cache = {}


def _view(ap, offset, dims):
    """AP view with explicit free dims; keeps the partition dim."""
    return bass.AP(
        tensor=ap.tensor,
        offset=ap.offset + offset,
        ap=[list(ap.ap[0])] + [list(d) for d in dims],
    )


def build_nc():
    if "nc" in _nc_cache:
        return _nc_cache["nc"]
    nc = bacc.Bacc(trn_type="TRN2", num_devices=NCORES, debug=False)

    xT_d = nc.dram_tensor("xT", [DIM, ROWS], BF16, kind="ExternalInput").ap()
    zT_d = nc.dram_tensor("zT", [DIM, ROWS], BF16, kind="ExternalInput").ap()
    wq_d = nc.dram_tensor("wq", [DIM, INNER], BF16, kind="ExternalInput").ap()
    wkv_d = nc.dram_tensor("wkv", [DIM, 2 * INNER], BF16, kind="ExternalInput").ap()
    wo_d = nc.dram_tensor("wo", [INNER, DIM], BF16, kind="ExternalInput").ap()
    cq_d = nc.dram_tensor("cq", [ROWS, DH], BF16, kind="ExternalInput").ap()
    sq_d = nc.dram_tensor("sq", [ROWS, DH], BF16, kind="ExternalInput").ap()
    ck_d = nc.dram_tensor("ck", [ROWS, DH], BF16, kind="ExternalInput").ap()
    sk_d = nc.dram_tensor("sk", [ROWS, DH], BF16, kind="ExternalInput").ap()
    yT_d = nc.dram_tensor("yT", [DIM, ROWS], BF16, kind="ExternalOutput").ap()
    cc_in = nc.dram_tensor("cc_in", [128, INNER], F32)
    cc_out = nc.dram_tensor("cc_out", [128, INNER], F32)

    with tile.TileContext(nc) as tc, ExitStack() as ctx:
        consts = ctx.enter_context(tc.tile_pool(name="consts", bufs=1))

        ident_f = consts.tile([128, 128], F32)
        make_identity(nc, ident_f)
        ident = consts.tile([128, 128], BF16)
        nc.vector.tensor_copy(ident, ident_f)
        eps_t = consts.tile([128, 1], F32)
        nc.vector.memset(eps_t, EPS)

        # phase-A-critical consts first on the sync queue; B-side consts
        # ride other engine queues so they stream in behind A's compute
        zT_sb, xT_sb, wq_t, wkv_t = [], [], [], []
        # first tiles' tables land before the bulk loads so tile 0's
        # vector work starts as soon as its kv matmul completes
        ckE = [consts.tile([128, DH], BF16, tag=f"ckE{t}", name=f"ckE{t}")
               for t in range(4)]
        skE = [consts.tile([128, DH], BF16, tag=f"skE{t}", name=f"skE{t}")
               for t in range(4)]
        for t in range(4):
            nc.sync.dma_start(ckE[t], ck_d[ts(t, 128), :])
            nc.sync.dma_start(skE[t], sk_d[ts(t, 128), :])
        for c in range(2):
            t = consts.tile([128, 2 * INNER], BF16, tag=f"wkv{c}")
            nc.sync.dma_start(t, wkv_d[ts(c, 128), :])
            wkv_t.append(t)
        NCH = 4
        CH = ROWS // NCH
        for c in range(2):
            zT_sb.append(
                [consts.tile([128, CH], BF16, tag=f"zT{c}_{j}", name=f"zT{c}_{j}")
                 for j in range(NCH)]
            )
            xT_sb.append(
                [consts.tile([128, CH], BF16, tag=f"xT{c}_{j}", name=f"xT{c}_{j}")
                 for j in range(NCH)]
            )
        for j in range(NCH):
            for c in range(2):
                nc.sync.dma_start(zT_sb[c][j], zT_d[ts(c, 128), ts(j, CH)])
        for c in range(2):
            t = consts.tile([128, INNER], BF16, tag=f"wq{c}")
            nc.scalar.dma_start(t, wq_d[ts(c, 128), :])
            wq_t.append(t)
        for j in range(NCH):
            for c in range(2):
                nc.scalar.dma_start(xT_sb[c][j], xT_d[ts(c, 128), ts(j, CH)])

        def zchunk(c, t):
            return zT_sb[c][t // (NT // NCH)][:, ts(t % (NT // NCH), 128)]

        def xchunk(c, t):
            return xT_sb[c][t // (NT // NCH)][:, ts(t % (NT // NCH), 128)]
        wo_t = []
        for p in range(4):
            t = consts.tile([128, DIM], BF16, tag=f"wo{p}")
            nc.gpsimd.dma_start(t, wo_d[ts(p, 128), :])
            wo_t.append(t)
        # qT staging: tile t occupies cols [512t, 512t+512), f-chunk f at
        # [512t+128f, 512t+128f+128). yT matmuls read strided views.
        qTb = consts.tile([128, NT * INNER], BF16)

        # ---------------- phase A: z side -> dotsT ----------------
        with tc.tile_pool(name="kv_pp", bufs=2, space="PSUM") as kv_pp, \
             tc.tile_pool(name="dots_pp", bufs=1, space="PSUM") as dots_pp, \
             tc.tile_pool(name="tabsA", bufs=8) as tabsA, \
             tc.tile_pool(name="elemA", bufs=8) as elemA, \
             tc.tile_pool(name="statsA", bufs=10) as statsA:

            dots_ps = []
            for p in range(4):
                dots_ps.append(
                    dots_pp.tile([128, 512], F32, tag=f"dots{p}", name=f"dots{p}")
                )

            for t in range(NT):
                if t < 4:
                    ck_t, sk_t = ckE[t], skE[t]
                else:
                    ck_t = tabsA.tile([128, DH], BF16, tag="ck")
                    nc.sync.dma_start(ck_t, ck_d[ts(t, 128), :])
                    sk_t = tabsA.tile([128, DH], BF16, tag="sk")
                    nc.sync.dma_start(sk_t, sk_d[ts(t, 128), :])

                kv_ps = kv_pp.tile([128, 2 * INNER], F32, tag="kv")
                for c in range(2):
                    nc.tensor.matmul(
                        kv_ps[:, 0:INNER],
                        zchunk(c, t),
                        wkv_t[c][:, 0:INNER],
                        start=(c == 0),
                        stop=(c == 1),
                    )
                for c in range(2):
                    nc.tensor.matmul(
                        kv_ps[:, INNER : 2 * INNER],
                        zchunk(c, t),
                        wkv_t[c][:, INNER : 2 * INNER],
                        start=(c == 0),
                        stop=(c == 1),
                    )

                # single PSUM reader: convert then everything reads SBUF bf16
                kvb_t = elemA.tile([128, 2 * INNER], BF16, tag="kvb")
                nc.scalar.copy(kvb_t, kv_ps)

                sq_t = elemA.tile([128, 2 * INNER], BF16, tag="sq")
                nc.scalar.activation(sq_t, kvb_t, AF.Square)
                red_t = statsA.tile([128, 16], F32, tag="red")
                nc.vector.reduce_sum(
                    red_t, _view(sq_t, 0, [[64, 16], [1, 64]]), axis=AX.X
                )
                std_t = statsA.tile([128, 16], F32, tag="std")
                nc.scalar.activation(
                    std_t, red_t, AF.Sqrt, scale=1.0 / DH, bias=eps_t[:, 0:1]
                )
                rstd_t = statsA.tile([128, 16], F32, tag="rstd")
                nc.vector.reciprocal(rstd_t, std_t)
                rkv_t = statsA.tile([128, 8], F32, tag="rkv")
                nc.vector.tensor_tensor(
                    rkv_t, rstd_t[:, 0:8], rstd_t[:, 8:16], op=ALU.mult
                )

                kc_t = elemA.tile([128, INNER], BF16, tag="kc")
                nc.vector.tensor_tensor(
                    kc_t,
                    kvb_t[:, 0:INNER],
                    _view(ck_t, 0, [[0, 8], [1, 64]]),
                    op=ALU.mult,
                )
                ks_t = elemA.tile([128, INNER], BF16, tag="ks")
                nc.gpsimd.tensor_tensor(
                    _view(ks_t, 0, [[64, 8], [32, 2], [1, 32]]),
                    _view(kvb_t, 32, [[64, 8], [-32, 2], [1, 32]]),
                    _view(sk_t, 0, [[0, 8], [32, 2], [1, 32]]),
                    op=ALU.mult,
                )
                v2_t = elemA.tile([128, INNER], BF16, tag="v2")
                nc.gpsimd.tensor_tensor(
                    v2_t,
                    kvb_t[:, INNER : 2 * INNER],
                    _view(rkv_t, 0, [[1, 8], [0, 64]]),
                    op=ALU.mult,
                )

                for p in range(4):
                    nc.tensor.matmul(
                        dots_ps[p][:, 0:128],
                        v2_t[:, ts(p, 128)],
                        kc_t[:, ts(p, 128)],
                        start=(t == 0),
                        stop=False,
                    )
                    nc.tensor.matmul(
                        dots_ps[p][:, 0:128],
                        v2_t[:, ts(p, 128)],
                        ks_t[:, ts(p, 128)],
                        start=False,
                        stop=(t == NT - 1),
                    )

            dotsT_sb = consts.tile([128, INNER], F32)
            for p in range(4):
                nc.vector.tensor_copy(dotsT_sb[:, ts(p, 128)], dots_ps[p][:, 0:128])

        # collective rides the gpsimd queue (idle during phase B) so its
        # DMAs don't block phase B's table loads on the sync queue
        nc.gpsimd.dma_start(cc_in.ap(), dotsT_sb)
        nc.gpsimd.collective_compute(
            "AllReduce",
            ALU.add,
            replica_groups=[[0, 1], [2, 3], [4, 5], [6, 7]],
            ins=[cc_in.ap()],
            outs=[cc_out.ap()],
        )
        dots_rd = consts.tile([128, INNER], F32)
        nc.gpsimd.dma_start(dots_rd, cc_out.ap())

        # ------- phase B: q side (overlaps AllReduce) + interleaved yT -------
        with tc.tile_pool(name="q_pp", bufs=2, space="PSUM") as q_pp, \
             tc.tile_pool(name="qt_pp", bufs=2, space="PSUM") as qt_pp, \
             tc.tile_pool(name="w2_pp", bufs=2, space="PSUM") as w2_pp, \
             tc.tile_pool(name="y_pp", bufs=2, space="PSUM") as y_pp, \
             tc.tile_pool(name="tabsB", bufs=8) as tabsB, \
             tc.tile_pool(name="elemB", bufs=8) as elemB, \
             tc.tile_pool(name="yout", bufs=6) as yout:

            W2b = [
                consts.tile([128, DIM], BF16, tag=f"W2b{p}", name=f"W2b{p}")
                for p in range(4)
            ]

            def b_tile(t):
                cq_t = tabsB.tile([128, DH], BF16, tag="cq")
                nc.sync.dma_start(cq_t, cq_d[ts(t, 128), :])
                sq2_t = tabsB.tile([128, DH], BF16, tag="sq2")
                nc.sync.dma_start(sq2_t, sq_d[ts(t, 128), :])

                q_ps = q_pp.tile([128, INNER], F32, tag="q")
                for c in range(2):
                    nc.tensor.matmul(
                        q_ps,
                        xchunk(c, t),
                        wq_t[c],
                        start=(c == 0),
                        stop=(c == 1),
                    )
                qb_t = elemB.tile([128, INNER], BF16, tag="qb")
                nc.scalar.copy(qb_t, q_ps)
                qc_t = elemB.tile([128, INNER], BF16, tag="qc")
                nc.vector.tensor_tensor(
                    qc_t, qb_t, _view(cq_t, 0, [[0, 8], [1, 64]]), op=ALU.mult
                )
                qs_t = elemB.tile([128, INNER], BF16, tag="qs")
                nc.vector.tensor_tensor(
                    _view(qs_t, 0, [[64, 8], [32, 2], [1, 32]]),
                    _view(qb_t, 32, [[64, 8], [-32, 2], [1, 32]]),
                    _view(sq2_t, 0, [[0, 8], [32, 2], [1, 32]]),
                    op=ALU.mult,
                )
                qr_t = elemB.tile([128, INNER], BF16, tag="qr")
                nc.vector.tensor_tensor(qr_t, qc_t, qs_t, op=ALU.add)

                qT_ps = qt_pp.tile([128, INNER], BF16, tag="qT")
                for f in range(4):
                    nc.tensor.transpose(
                        qT_ps[:, ts(f, 128)], qr_t[:, ts(f, 128)], ident
                    )
                nc.scalar.copy(qTb[:, ts(t, INNER)], qT_ps)

            def yt_group(r):
                for o in range(2):
                    yT_ps = y_pp.tile([128, 512], F32, tag="yT")
                    for f in range(4):
                        nc.tensor.matmul(
                            yT_ps,
                            W2b[f][:, ts(o, 128)],
                            _view(qTb, r * 4 * INNER + f * 128,
                                  [[INNER, 4], [1, 128]]),
                            start=(f == 0),
                            stop=(f == 3),
                        )
                    yT_sb = yout.tile([128, 512], BF16, tag="ysb")
                    if o == 0:
                        nc.vector.tensor_copy(yT_sb, yT_ps)
                    else:
                        nc.scalar.copy(yT_sb, yT_ps)
                    nc.sync.dma_start(yT_d[ts(o, 128), ts(r, 512)], yT_sb)

            for t in range(24):
                b_tile(t)

            # W2 = blkdiag(dotsT) @ WoT (collective done by now)
            blk = consts.tile([128, INNER], BF16)
            nc.vector.memset(blk, 0.0)
            nc.vector.tensor_copy(
                _view(blk[0:64, 0:1], 0, [[128, 4], [1, 64]]),
                _view(dots_rd[0:64, 0:1], 0, [[128, 4], [1, 64]]),
            )
            nc.vector.tensor_copy(
                _view(blk[64:128, 0:1], 64, [[128, 4], [1, 64]]),
                _view(dots_rd[64:128, 0:1], 64, [[128, 4], [1, 64]]),
            )
            for p in range(4):
                w2_ps = w2_pp.tile([128, 512], F32, tag="w2")
                nc.tensor.matmul(
                    w2_ps[:, 0:DIM], blk[:, ts(p, 128)], wo_t[p],
                    start=True, stop=True,
                )
                nc.scalar.copy(W2b[p], w2_ps[:, 0:DIM])

            for r in range(5):
                yt_group(r)
            for t in range(24, 28):
                b_tile(t)
            yt_group(5)
            for t in range(28, 32):
                b_tile(t)
            yt_group(6)
            yt_group(7)

    nc.compile()
    _nc_cache["nc"] = nc
    return nc


PERM64 = np.concatenate(
    [np.arange(0, 16), np.arange(32, 48), np.arange(16, 32), np.arange(48, 64)]
)
PERM = (np.arange(H)[:, None] * DH + PERM64[None, :]).reshape(-1)


def _tables64(pos):
    """pos [rows, 2] -> cos/sin [rows, 64] in permuted within-head order;
    sin sign-baked (- for first halves)."""
    rdim = DH // 2
    inv_freq = (
        1.0 / (10000.0 ** (np.arange(0, rdim, 2, dtype=np.float32) / rdim))
    ).astype(np.float32)
    t = pos.astype(np.float32) * np.float32(64.0)  # SCALE / MIN_FREQ
    fx = t[:, 0:1] * inv_freq
    fy = t[:, 1:2] * inv_freq
    f = np.concatenate([fx, fy, fx, fy], axis=1)
    c = np.cos(f)
    s = np.sin(f)
    s[:, :32] *= -1.0
    return c.astype(BF), s.astype(BF)


def _prepare(x, z, x_pos, z_pos, Wq, Wkv, k_gamma, k_beta, v_gamma, v_beta, Wo, bo):
    triv = (
        np.all(np.asarray(k_gamma) == 1.0)
        and np.all(np.asarray(k_beta) == 0.0)
        and np.all(np.asarray(v_gamma) == 1.0)
        and np.all(np.asarray(v_beta) == 0.0)
    )
    assert triv, "non-trivial gamma/beta not supported by this kernel"

    xf = np.asarray(x, dtype=np.float32).reshape(B * N1, DIM)
    zf = np.asarray(z, dtype=np.float32).reshape(B * N1, DIM)
    xpf = np.asarray(x_pos).reshape(B * N1, 2)
    zpf = np.asarray(z_pos).reshape(B * N1, 2)

    wqT = ((np.asarray(Wq).T / np.float32(N1))[:, PERM]).astype(BF)
    wkvT = np.asarray(Wkv).T.astype(np.float32)  # [256, 1024]
    wkv_c = wkvT.reshape(DIM, 16, DH)
    wkvT = (wkv_c - wkv_c.mean(axis=2, keepdims=True)).reshape(DIM, 2 * INNER)
    wkvT = np.concatenate(
        [wkvT[:, :INNER][:, PERM], wkvT[:, INNER:]], axis=1
    ).astype(BF)
    woT = np.ascontiguousarray(np.asarray(Wo).T).astype(BF)  # [512, 256]

    cq, sq_ = _tables64(xpf)
    ck, sk_ = _tables64(zpf)

    nc = build_nc()
    in_maps = []
    for c in range(NCORES):
        lo, hi = c * ROWS, (c + 1) * ROWS
        in_maps.append(
            {
                "xT": np.ascontiguousarray(xf[lo:hi].T).astype(BF),
                "zT": np.ascontiguousarray(zf[lo:hi].T).astype(BF),
                "wq": np.ascontiguousarray(wqT),
                "wkv": np.ascontiguousarray(wkvT),
                "wo": woT,
                "cq": np.ascontiguousarray(cq[lo:hi]),
                "sq": np.ascontiguousarray(sq_[lo:hi]),
                "ck": np.ascontiguousarray(ck[lo:hi]),
                "sk": np.ascontiguousarray(sk_[lo:hi]),
            }
        )
    return nc, in_maps


def _finalize(results, bo):
    ys = [
        np.asarray(results[c]["yT"]).astype(np.float32).T for c in range(NCORES)
    ]  # each [4096, 256]
    y = np.concatenate(ys, axis=0) + np.asarray(bo, dtype=np.float32)
    return y.reshape(B, N1, DIM).astype(np.float32)


def kernel(**inputs):
    nc, in_maps = _prepare(**inputs)
    res = run_bass_kernel_spmd(nc, in_maps, list(range(NCORES)))
    return _finalize(res.results, inputs["bo"])
